# revision 12
# baseline (speedup 1.0000x reference)
"""Trainium2 Bass kernel for nn_Channel_dot — hybrid fp8-DoubleRow/bf16.

Math (per batch b):
  x1 = reshape(input1) -> [THW, C];  x2 likewise
  q  = W1 @ x1 + b1            [F, C]
  k  = W2 @ x2 + b2            [F, C]
  sT = k^T q                   [C(d), C(c)]  (sT[d,c] = s[c,d])
  scoresT = softmax over c (free axis of sT)   -- fp32
  out[c,o] = sum_d s[c,d] * (W3 @ x1 + b3)[o,d]
           = sum_i uT[i,c] * W3T[i,o] + r[c]*b3[o]
    where uT[i,c] = sum_d x1[i,d] sT[d,c],  r[c] = sum_d sT[d,c]

Sharding: 8 cores = 4 batches x 2 halves of the G3 output dim (O=16384).

o-groups 0/1 run the direct order (gT = x1^T W3T streamed, then s @ gT)
in pure bf16 — they keep the PE busy while x1/x2 stream in and
q/k/softmax resolve.  o-groups 2..15 use the reassociated order, with
the first 2*P8 i-chunks (of 40) as P8 fp8e4 DoubleRow matmuls (2 chunks
per MM at the same 216ns as one bf16 MM — measured on HW) and the rest
bf16.  P8 is calibrated so global rel-err stays under the 2e-2 gate.

Scale algebra (powers of two, exact): sT is produced ALPHA-scaled, so u
arrives in PSUM as ALPHA*u and casts straight to fp8e4 (clipped +-224
via the two tensor_scalar alu slots); W3 is staged BETA-scaled (fp8
rows e4m3, bf16 rows an exact exponent shift) so fp8 and bf16 chunks
accumulate in ONE psum bank at scale ALPHA*BETA.  og0/1 use GAMMA-
scaled x1; the host divides the output by the per-region scale at the
end.
"""

import os
import sys

for _p in ("/opt/trn_rl_repo", "/root/.axon_site/_ro/trn_rl_repo"):
    if os.path.isdir(_p) and _p not in sys.path:
        sys.path.insert(0, _p)

import numpy as np
import ml_dtypes

import concourse.bacc as bacc
import concourse.bass as bass
import concourse.mybir as mybir
import concourse.tile as tile
from concourse.bass_utils import run_bass_kernel_spmd

B, T, C, H, W = 4, 5, 512, 32, 32
F = 16
THW = T * H * W            # 5120
O_TOT = F * H * W          # 16384
O_HALF = O_TOT // 2        # 8192 per core
NI = THW // 128            # 40 i-chunks
OG = 512                   # o-columns per inner group (1 PSUM bank)
N_OG = O_HALF // OG        # 16
N_MOG = N_OG - 2           # reassociated (main) o-groups
NDT = C // 128             # 4 channel tiles

P8 = 8                     # fp8 pair-chunks per (main og, ct)
NF8 = 2 * P8               # i-chunks covered by fp8
NBF = NI - NF8             # bf16 i-chunks in main o-groups
ALPHA = 2.0 ** 5           # sT scale (alpha*u max ~210 < 240)
BETA = 2.0 ** 13           # W3 scale (beta*W3 max ~115 < 240)
BETA01 = 2.0 ** 8          # W3 scale for og0/1 (g_sb must fit fp16)
GAMMA = 2.0 ** 5           # x1 scale for og0/1 direct path

f32 = mybir.dt.float32
bf16 = mybir.dt.bfloat16
fp16 = mybir.dt.float16
fp8e4 = mybir.dt.float8e4
DR = mybir.MatmulPerfMode.DoubleRow
AF = mybir.ActivationFunctionType
AX = mybir.AxisListType
ALU = mybir.AluOpType
BF16NP = np.dtype(ml_dtypes.bfloat16)
FP16NP = np.dtype(np.float16)
E4M3NP = np.dtype(ml_dtypes.float8_e4m3)

_NC_CACHE = {}


def _chunk_plan(total, step):
    # near-equal parts, each <= step (DMA descriptor batching)
    k = -(-total // step)
    base, rem = divmod(total, k)
    return [base + (i < rem) for i in range(k)]


def _build_nc():
    # Bacc (not plain Bass): its finalize() runs generate_event_semaphores(),
    # which splits multi-wait sync onto EventSemaphore ops — TRN2 compute
    # instructions encode at most one sync wait.
    nc = bacc.Bacc()

    # All streamed inputs are staged partition-major on the host so each
    # DMA reads multi-KB contiguous runs per partition.
    NCH = 5                     # i-chunks per DMA chunk
    x1 = nc.dram_tensor("x1", [128, NI * C], fp16, kind="ExternalInput")
    x2 = nc.dram_tensor("x2", [128, NI * C], fp16, kind="ExternalInput")
    # x1 transposed to channel-major: x1t[p, (n, dt, j)] = x1[i=n*128+j,
    # c=dt*128+p] — the u-phase contracts over the channel dim.
    x1t = nc.dram_tensor("x1t", [128, NI * NDT * 128], fp16, kind="ExternalInput")
    w1t = nc.dram_tensor("w1t", [128, NI * F], fp16, kind="ExternalInput")
    w2t = nc.dram_tensor("w2t", [128, NI * F], fp16, kind="ExternalInput")
    # W3 streams: og0/1 full-bf16; main ogs split fp8-pairs + bf16 tail.
    w3g = nc.dram_tensor("w3g", [128, 2 * NI * OG], fp16, kind="ExternalInput")
    w3q = nc.dram_tensor("w3q", [128, N_MOG * P8 * 2 * OG], fp8e4,
                         kind="ExternalInput")
    w3b = nc.dram_tensor("w3b", [128, N_MOG * NBF * OG], fp16,
                         kind="ExternalInput")
    b1 = nc.dram_tensor("b1", [F, 1], f32, kind="ExternalInput")
    b2 = nc.dram_tensor("b2", [F, 1], f32, kind="ExternalInput")
    # b3 replicated to 128 partitions on the host; streamed per o-group.
    b3b = nc.dram_tensor("b3b", [128, O_HALF], bf16, kind="ExternalInput")
    b3g = nc.dram_tensor("b3g", [128, 2 * OG], fp16, kind="ExternalInput")
    out = nc.dram_tensor("out", [C, O_HALF], f32, kind="ExternalOutput")

    x1_r = x1.rearrange("p (n c) -> p n c", c=C)
    x2_r = x2.rearrange("p (n c) -> p n c", c=C)
    x1t_r = x1t.rearrange("p (n dt j) -> p n dt j", dt=NDT, j=128)
    w3g_r = w3g.rearrange("p (og n oc) -> p og n oc", og=2, n=NI)
    w3q_r = w3q.rearrange("p (og j s oc) -> p og j s oc", og=N_MOG, j=P8, s=2)
    w3b_r = w3b.rearrange("p (og m oc) -> p og m oc", og=N_MOG, m=NBF)
    w1_r = w1t.rearrange("p (n f) -> p n f", f=F)
    w2_r = w2t.rearrange("p (n f) -> p n f", f=F)
    out_r = out.rearrange("(ct p) o -> ct p o", p=128)

    with tile.TileContext(nc) as tc:
        with (
            tc.tile_pool(name="persist", bufs=1) as persist,
            tc.tile_pool(name="w3p", bufs=6) as w3p,
            tc.tile_pool(name="w3qp", bufs=3) as w3qp,
            tc.tile_pool(name="gsbp", bufs=2) as gsbp,
            tc.tile_pool(name="outp", bufs=4) as outp,
            tc.tile_pool(name="b3p", bufs=3) as b3p,
            tc.tile_pool(name="tmpp", bufs=8) as tmpp,
            tc.tile_pool(name="small", bufs=4) as small,
            tc.tile_pool(name="pg", bufs=5, space="PSUM") as pg,
            tc.tile_pool(name="po", bufs=2, space="PSUM") as po,
            tc.tile_pool(name="pqk", bufs=1, space="PSUM") as pqk,
        ):
            # ---- persistent tiles ----
            # x1 (i-major) is dead after og0/1's g-streams + q; the
            # x1T layout reuses its SBUF slot via the shared tag.
            x1_sb = persist.tile([128, NI, C], fp16, name="x1_sb", tag="x1x")
            x1t_sb = persist.tile(
                [128, NI, NDT, 128], fp16, name="x1t_sb", tag="x1x"
            )
            # x2 is dead after k; uT (bf16 + fp8 parts) reuses its slot.
            x2_sb = persist.tile([128, NI, C], fp16, name="x2_sb", tag="xu")
            ut_sb = persist.tile([128, NBF, C], fp16, name="ut_sb", tag="xu")
            ut8_sb = persist.tile([128, P8, 2, C], fp8e4, name="ut8_sb")
            sT_sb = persist.tile([128, NDT, C], fp16, name="sT_sb")

            def b3_tile(og):
                dt_b3 = fp16 if og < 2 else bf16
                b3_t = b3p.tile([128, OG], dt_b3, name="b3_t")
                src_t = b3g if og < 2 else b3b
                nc.sync.dma_start(out=b3_t[:], in_=src_t[:, og * OG : (og + 1) * OG])
                return b3_t

            def w3q_fetch(og):
                w3q_t = w3qp.tile([128, P8, 2, OG], fp8e4, name="w3q_t")
                nc.sync.dma_start(out=w3q_t[:], in_=w3q_r[:, og - 2])
                return w3q_t

            def g_phase(og, x_load=None):
                """Direct-order o-group (0/1): stream W3 columns, accumulate
                gT = (GAMMA x1)^T (BETA W3T) in PSUM, pure bf16."""
                g_ps_l = [pg.tile([128, OG], f32, name="g_ps") for _ in range(NDT)]
                # og 0 ramps with fine-grained chunks so the very first
                # matmul starts as early as possible (DMA queues are still
                # spinning up during the first ~15us)
                plan = [1, 1, 2, 3, 4, 4, 5, 5, 5, 5, 5] if og == 0 else \
                    _chunk_plan(NI, NCH)
                n0 = 0
                for ch in plan:
                    if x_load is not None:
                        # one x chunk rides along per w3 chunk so the
                        # prologue inputs arrive without their own phase
                        nc.sync.dma_start(
                            out=x_load[0][:, n0 : n0 + ch, :],
                            in_=x_load[1][:, n0 : n0 + ch, :],
                        )
                    w3_t = w3p.tile([128, NCH, OG], fp16, name="w3_t")
                    nc.sync.dma_start(
                        out=w3_t[:, :ch, :], in_=w3g_r[:, og, n0 : n0 + ch, :]
                    )
                    for j in range(ch):
                        for dt_ in range(NDT):
                            nc.tensor.matmul(
                                g_ps_l[dt_][:],
                                lhsT=x1_sb[:, n0 + j, dt_ * 128 : (dt_ + 1) * 128],
                                rhs=w3_t[:, j, :],
                                start=(n0 + j == 0),
                                stop=(n0 + j == NI - 1),
                            )
                    n0 += ch
                return g_ps_l

            def evac_phase(g_ps_l, b3_t):
                """Evacuate gT (+b3) to SBUF right after its g-stream ends,
                while the Vector engine is idle."""
                g_sb = gsbp.tile([128, NDT, OG], fp16, name="g_sb")
                for dt_ in range(NDT):
                    nc.vector.tensor_add(
                        g_sb[:, dt_, :], g_ps_l[dt_][:], b3_t[:]
                    )
                return g_sb

            def out_phase(og, g_sb):
                """scores @ gT for a direct-order o-group."""
                osl = slice(og * OG, (og + 1) * OG)
                for ct in range(NDT):
                    o_ps = po.tile([128, OG], f32, name="o_ps", tag="so")
                    for dt_ in range(NDT):
                        nc.tensor.matmul(
                            o_ps[:],
                            lhsT=sT_sb[:, dt_, ct * 128 : (ct + 1) * 128],
                            rhs=g_sb[:, dt_, :],
                            start=(dt_ == 0),
                            stop=(dt_ == NDT - 1),
                        )
                    out_t = outp.tile([128, OG], f32, name="out_t")
                    nc.vector.tensor_copy(out_t[:], o_ps[:])
                    nc.sync.dma_start(out=out_r[ct, :, osl], in_=out_t[:])

            def bog_begin(og):
                """First half of a reassociated o-group: bias prep + the P8
                fp8-DoubleRow MMs (only need ut8 + the prefetched w3q)."""
                b3_t = b3_tile(og)
                # rank-1 bias term precomputed on DVE; consumed by the
                # PSUM evacuation adds at the end of the stream
                tmp_l = []
                for ct in range(NDT):
                    tmp_t = tmpp.tile([128, OG], bf16, name="tmp_t")
                    nc.vector.tensor_scalar_mul(
                        tmp_t[:], b3_t[:], r_sb[:, ct : ct + 1]
                    )
                    tmp_l.append(tmp_t)
                ps_l = [pg.tile([128, OG], f32, name="g_ps") for _ in range(NDT)]
                w3q_t = w3q_pending.pop(og, None)
                if w3q_t is None:
                    w3q_t = w3q_fetch(og)
                for j in range(P8):
                    for ct in range(NDT):
                        nc.tensor.matmul(
                            ps_l[ct][:],
                            lhsT=ut8_sb[:, j, :, ct * 128 : (ct + 1) * 128],
                            rhs=w3q_t[:, j, :, :],
                            start=(j == 0),
                            stop=False,
                            perf_mode=DR,
                        )
                return ps_l, tmp_l

            def bog_finish(og, ps_l, tmp_l):
                """Second half: the NBF fp16 MMs + evacuation.  The last
                chunk group runs ct-major with per-ct stop + immediate
                evacuation, so three of the four PSUM banks free while the
                tail MMs still stream — the next o-group's first matmuls
                then never wait on this one's evacuation DVE ops."""
                osl = slice(og * OG, (og + 1) * OG)
                mog = og - 2
                plan = _chunk_plan(NBF, NCH)
                m0 = 0
                for gi, ch in enumerate(plan):
                    w3_t = w3p.tile([128, NCH, OG], fp16, name="w3_t")
                    nc.sync.dma_start(
                        out=w3_t[:, :ch, :], in_=w3b_r[:, mog, m0 : m0 + ch, :]
                    )
                    if gi < len(plan) - 1:
                        for j in range(ch):
                            for ct in range(NDT):
                                nc.tensor.matmul(
                                    ps_l[ct][:],
                                    lhsT=ut_sb[:, m0 + j, ct * 128 : (ct + 1) * 128],
                                    rhs=w3_t[:, j, :],
                                    start=False,
                                    stop=False,
                                )
                    else:
                        for ct in range(NDT):
                            for j in range(ch):
                                nc.tensor.matmul(
                                    ps_l[ct][:],
                                    lhsT=ut_sb[:, m0 + j, ct * 128 : (ct + 1) * 128],
                                    rhs=w3_t[:, j, :],
                                    start=False,
                                    stop=(j == ch - 1),
                                )
                            out_t = outp.tile([128, OG], f32, name="out_t")
                            nc.vector.tensor_add(
                                out_t[:], ps_l[ct][:], tmp_l[ct][:]
                            )
                            nc.sync.dma_start(out=out_r[ct, :, osl], in_=out_t[:])
                    m0 += ch

            def bog_phase_ct_major(og):
                """Last o-group runs ct-major: each row tile's accumulation
                completes before the next starts, so its evacuation + out
                DMA overlap the remaining tiles' matmuls (shorter tail)."""
                osl = slice(og * OG, (og + 1) * OG)
                mog = og - 2
                b3_t = b3_tile(og)
                tmp_l = []
                for ct in range(NDT):
                    tmp_t = tmpp.tile([128, OG], bf16, name="tmp_t")
                    nc.vector.tensor_scalar_mul(
                        tmp_t[:], b3_t[:], r_sb[:, ct : ct + 1]
                    )
                    tmp_l.append(tmp_t)
                ps_l = [pg.tile([128, OG], f32, name="g_ps") for _ in range(NDT)]
                w3q_t = w3q_pending.pop(og, None)
                if w3q_t is None:
                    w3q_t = w3q_fetch(og)
                w3_tl = []
                m0 = 0
                for ch in _chunk_plan(NBF, NCH):
                    w3_t = w3p.tile([128, NCH, OG], fp16, name="w3_t")
                    nc.sync.dma_start(
                        out=w3_t[:, :ch, :], in_=w3b_r[:, mog, m0 : m0 + ch, :]
                    )
                    w3_tl.append((w3_t, m0, ch))
                    m0 += ch
                for ct in range(NDT):
                    for j in range(P8):
                        nc.tensor.matmul(
                            ps_l[ct][:],
                            lhsT=ut8_sb[:, j, :, ct * 128 : (ct + 1) * 128],
                            rhs=w3q_t[:, j, :, :],
                            start=(j == 0),
                            stop=False,
                            perf_mode=DR,
                        )
                    for w3_t, m0, ch in w3_tl:
                        for j in range(ch):
                            nc.tensor.matmul(
                                ps_l[ct][:],
                                lhsT=ut_sb[:, m0 + j, ct * 128 : (ct + 1) * 128],
                                rhs=w3_t[:, j, :],
                                start=False,
                                stop=(m0 + j == NBF - 1),
                            )
                    out_t = outp.tile([128, OG], f32, name="out_t")
                    nc.vector.tensor_add(out_t[:], ps_l[ct][:], tmp_l[ct][:])
                    nc.sync.dma_start(out=out_r[ct, :, osl], in_=out_t[:])

            # o-group 0's g-stream first, with x1 loads interleaved: the PE
            # starts as soon as the first x1/W3 tile pair lands.
            g0 = g_phase(0, x_load=(x1_sb, x1_r))  # x1 rides og0's stream
            b3_t0 = b3_tile(0)

            # W1T/W2T zero-padded on-chip to 128 output columns: M=128
            # matmuls get fast weight load while only 160KB each moves.
            w1t_sb = persist.tile([128, NI, 128], fp16, name="w1t_sb")
            nc.vector.memset(w1t_sb[:], 0.0)
            nc.sync.dma_start(out=w1t_sb[:, :, :F], in_=w1_r[:])
            w2t_sb = persist.tile([128, NI, 128], fp16, name="w2t_sb")
            nc.vector.memset(w2t_sb[:], 0.0)
            nc.sync.dma_start(out=w2t_sb[:, :, :F], in_=w2_r[:])
            b1_sb = persist.tile([F, 1], f32, name="b1_sb")
            nc.sync.dma_start(out=b1_sb[:], in_=b1[:])
            b2_sb = persist.tile([F, 1], f32, name="b2_sb")
            nc.sync.dma_start(out=b2_sb[:], in_=b2[:])
            ones_sb = persist.tile([128, 1], fp16, name="ones_sb")
            nc.vector.memset(ones_sb[:], 1.0)

            # ---- q = (W1/GAMMA) @ (GAMMA x1) + b1 -> [F, C] fp32 ----
            q_ps = pqk.tile([128, C], f32, name="q_ps", tag="qk")
            for n in range(NI):
                nc.tensor.matmul(
                    q_ps[:],
                    lhsT=w1t_sb[:, n, :],
                    rhs=x1_sb[:, n, :],
                    start=(n == 0),
                    stop=(n == NI - 1),
                )
            q_sb = persist.tile([F, C], f32, name="q_sb")
            nc.vector.tensor_scalar_add(q_sb[:], q_ps[:F, :], b1_sb[:])

            # og0's gT evacuates now (Vector is idle; g0 psum is complete)
            g_sb0 = evac_phase(g0, b3_t0)

            # o-group 1's g-stream carries the x2 loads (k runs after it)
            g1 = g_phase(1, x_load=(x2_sb, x2_r))
            b3_t1 = b3_tile(1)

            # ---- k = W2 @ x2 + b2 -> [F, C] fp32 ----
            k_ps = pqk.tile([128, C], f32, name="k_ps", tag="qk")
            for n in range(NI):
                nc.tensor.matmul(
                    k_ps[:],
                    lhsT=w2t_sb[:, n, :],
                    rhs=x2_sb[:, n, :],
                    start=(n == 0),
                    stop=(n == NI - 1),
                )
            k_sb = persist.tile([F, C], f32, name="k_sb")
            nc.vector.tensor_scalar_add(k_sb[:], k_ps[:F, :], b2_sb[:])

            # og1's gT evacuates immediately too
            g_sb1 = evac_phase(g1, b3_t1)

            # x1T streams in while softmax/out-phases run; the u-phase
            # consumes it granule by granule.  og2/og3's fp8 W3 streams
            # prefetch here (interleaved) so og2's DoubleRow matmuls don't
            # stall on DMA right after the u-phase.
            w3q_pending = {}
            for gch in range(NI // NCH):
                nc.sync.dma_start(
                    out=x1t_sb[:, gch * NCH : (gch + 1) * NCH, :, :],
                    in_=x1t_r[:, gch * NCH : (gch + 1) * NCH, :, :],
                )
                if gch == 0:
                    w3q_pending[2] = w3q_fetch(2)
                elif gch == 1:
                    w3q_pending[3] = w3q_fetch(3)

            # ---- sT[d, c] = sum_f k[f,d] q[f,c] (plain fp32 matmul),
            #      then softmax over free (c); emit ALPHA-scaled bf16
            #      scores.  The tiny r-matmuls (r[c] = sum_d sT[d,c],
            #      partition reduce via a ones vector) interleave into the
            #      softmax window. ----
            # four separate PSUM tiles: a column-sliced accumulation in one
            # bank corrupts sibling columns (start=True resets the bank).
            r_ps_l = [pg.tile([128, 1], f32, name="g_ps") for _ in range(NDT)]

            def softmax_tail(dt_, s_ps):
                # logits are bounded (|s| < ~10 for this problem), so plain
                # exp is fp32-safe; skipping the max keeps Exp at one sync
                # wait (the Activation ISA slot allows only one).
                e_sb = small.tile([128, C], f32, name="e_sb")
                esum = small.tile([128, 1], f32, name="esum")
                nc.scalar.activation(
                    e_sb[:], s_ps[:], AF.Exp, scale=1.0, accum_out=esum[:],
                )
                rcp = small.tile([128, 1], f32, name="rcp")
                nc.vector.reciprocal(rcp[:], esum[:])
                rcp_a = small.tile([128, 1], f32, name="rcp_a")
                nc.vector.tensor_scalar_mul(rcp_a[:], rcp[:], float(ALPHA))
                nc.vector.tensor_scalar_mul(sT_sb[:, dt_, :], e_sb[:], rcp_a[:])
                for ct in range(NDT):
                    nc.tensor.matmul(
                        r_ps_l[ct][:],
                        lhsT=sT_sb[:, dt_, ct * 128 : (ct + 1) * 128],
                        rhs=ones_sb[:],
                        start=(dt_ == 0),
                        stop=(dt_ == NDT - 1),
                    )

            s_pend = None
            for dt_ in range(NDT):
                s_ps = po.tile([128, C], f32, name="s_ps", tag="so")
                nc.tensor.matmul(
                    s_ps[:],
                    lhsT=k_sb[:, dt_ * 128 : (dt_ + 1) * 128],
                    rhs=q_sb[:],
                    start=True,
                    stop=True,
                )
                if s_pend is not None:
                    softmax_tail(dt_ - 1, s_pend)
                s_pend = s_ps
            softmax_tail(NDT - 1, s_pend)
            r_sb = persist.tile([128, NDT], f32, name="r_sb")
            for ct in range(NDT):
                nc.vector.tensor_copy(r_sb[:, ct : ct + 1], r_ps_l[ct][:])

            # ---- direct-order output for o-groups 0/1 ----
            out_phase(0, g_sb0)
            out_phase(1, g_sb1)

            # ---- u-phase: uT[i, c] = ALPHA * sum_d x1[i,d] s[c,d] ----
            # first NF8 chunks quantize to fp8e4 (clipped +-224 via the two
            # tensor_scalar alu slots); the rest evacuate fp16.
            def u_chunk(n):
                u_ps = po.tile([128, C], f32, name="u_ps", tag="so")
                for dt_ in range(NDT):
                    nc.tensor.matmul(
                        u_ps[:],
                        lhsT=x1t_sb[:, n, dt_, :],
                        rhs=sT_sb[:, dt_, :],
                        start=(dt_ == 0),
                        stop=(dt_ == NDT - 1),
                    )
                if n < NF8:
                    nc.vector.tensor_scalar(
                        ut8_sb[:, n // 2, n % 2, :], u_ps[:],
                        224.0, -224.0, ALU.min, ALU.max,
                    )
                else:
                    nc.vector.tensor_copy(ut_sb[:, n - NF8, :], u_ps[:])

            for n in range(NF8):
                u_chunk(n)
            # og2's fp8-DoubleRow block interleaves here: its inputs (ut8 +
            # the prefetched w3q) are ready, and it gives the x1t stream
            # breathing room so the tail u-chunks don't stall on DMA.
            og2_ps, og2_tmp = bog_begin(2)
            for n in range(NF8, NI):
                u_chunk(n)
            bog_finish(2, og2_ps, og2_tmp)

            # ---- main: reassociated hybrid stream for o-groups 3..15 ----
            for og in range(3, N_OG - 1):
                ps_l, tmp_l = bog_begin(og)
                bog_finish(og, ps_l, tmp_l)
            bog_phase_ct_major(N_OG - 1)

    nc.finalize()
    return nc


def _get_nc():
    if "nc" not in _NC_CACHE:
        _NC_CACHE["nc"] = _build_nc()
    return _NC_CACHE["nc"]


def _stage_inputs(input1, input2, W1, b1, W2, b2, W3, b3):
    input1 = np.asarray(input1, np.float32)
    input2 = np.asarray(input2, np.float32)
    W1 = np.asarray(W1, np.float32)
    W2 = np.asarray(W2, np.float32)
    W3 = np.asarray(W3, np.float32)
    b1 = np.asarray(b1, np.float32)
    b2 = np.asarray(b2, np.float32)
    b3 = np.asarray(b3, np.float32)

    def pmajor(X, inner):
        # [THW, inner] -> [128, NI*inner]: row p = concat_n X[n*128+p, :]
        return np.ascontiguousarray(
            X.reshape(-1, 128, inner).transpose(1, 0, 2).reshape(128, -1)
        )

    # [B,T,C,H,W] -> x[b][i=(t,hw), c], partition-major
    X1f = np.ascontiguousarray(
        input1.reshape(B, T, C, H * W).transpose(0, 1, 3, 2)
    ).reshape(B, THW, C)
    X2f = np.ascontiguousarray(
        input2.reshape(B, T, C, H * W).transpose(0, 1, 3, 2)
    ).reshape(B, THW, C)
    X1g = (GAMMA * X1f).astype(FP16NP)          # gamma-scaled fp16
    X2 = X2f.astype(FP16NP)
    X1p = [pmajor(X1g[b], C) for b in range(B)]
    X2p = [pmajor(X2[b], C) for b in range(B)]
    # channel-major x1 (unscaled): [128p, (n, dt, j)] = x1[n*128+j, dt*128+p]
    X1b = X1f.astype(FP16NP)
    X1Tp = [
        np.ascontiguousarray(
            X1b[b].reshape(NI, 128, NDT, 128).transpose(3, 0, 2, 1)
        ).reshape(128, NI * NDT * 128)
        for b in range(B)
    ]
    W1Tp = pmajor(np.ascontiguousarray((W1 / GAMMA).T).astype(FP16NP), F)
    W2Tp = pmajor(np.ascontiguousarray(W2.T).astype(FP16NP), F)
    W3T = np.ascontiguousarray(W3.T)             # [THW, O_TOT]
    W3q8 = (BETA * W3T[: NF8 * 128]).astype(E4M3NP)   # fp8 rows (main ogs)
    W3bb = (BETA * W3T).astype(FP16NP)           # fp16 rows, main ogs
    W3gg = (BETA01 * W3T).astype(FP16NP)         # fp16 rows, og0/1
    b1c = np.ascontiguousarray(b1.reshape(F, 1))
    b2c = np.ascontiguousarray(b2.reshape(F, 1))

    in_maps = []
    for core in range(8):
        b = core // 2
        half = core % 2
        osl = slice(half * O_HALF, (half + 1) * O_HALF)
        W3h8 = W3q8[:, osl]
        W3hb = W3bb[:, osl]
        W3hg = W3gg[:, osl]
        # og0/1: full-depth fp16 [128, (og2, n40, oc)]
        w3g_core = np.ascontiguousarray(
            W3hg[:, : 2 * OG]
            .reshape(NI, 128, 2, OG)
            .transpose(1, 2, 0, 3)
            .reshape(128, 2 * NI * OG)
        )
        # main ogs fp8 pairs -> [128, (og, j, s, oc)]
        w3q_core = np.ascontiguousarray(
            W3h8[:, 2 * OG :]
            .reshape(P8, 2, 128, N_MOG, OG)
            .transpose(2, 3, 0, 1, 4)
            .reshape(128, N_MOG * P8 * 2 * OG)
        )
        # main ogs bf16 tail -> [128, (og, m, oc)]
        w3b_core = np.ascontiguousarray(
            W3hb[NF8 * 128 :, 2 * OG :]
            .reshape(NBF, 128, N_MOG, OG)
            .transpose(1, 2, 0, 3)
            .reshape(128, N_MOG * NBF * OG)
        )
        b3h = b3[osl]
        in_maps.append(
            {
                "x1": X1p[b],
                "x2": X2p[b],
                "x1t": X1Tp[b],
                "w1t": W1Tp,
                "w2t": W2Tp,
                "w3g": w3g_core,
                "w3q": w3q_core,
                "w3b": w3b_core,
                "b1": b1c,
                "b2": b2c,
                "b3b": np.ascontiguousarray(
                    np.broadcast_to(
                        (BETA * b3h).astype(BF16NP)[None, :], (128, O_HALF)
                    )
                ),
                "b3g": np.ascontiguousarray(
                    np.broadcast_to(
                        (GAMMA * BETA01 * b3h[: 2 * OG]).astype(FP16NP)[None, :],
                        (128, 2 * OG),
                    )
                ),
            }
        )
    return in_maps


def run(inputs: dict, trace: bool = False):
    """Returns (full_output [B,F,C,H,W], BassKernelResults)."""
    in_maps = _stage_inputs(**inputs)
    nc = _get_nc()
    res = run_bass_kernel_spmd(nc, in_maps, core_ids=list(range(8)), trace=trace)
    out_full = np.empty((B, C, O_TOT), np.float32)
    for core in range(8):
        b = core // 2
        half = core % 2
        out_full[b, :, half * O_HALF : (half + 1) * O_HALF] = res.results[core]["out"]
    # host unscale: og0/1 of each half at ALPHA*GAMMA*BETA, rest ALPHA*BETA
    inv_main = 1.0 / (ALPHA * BETA)
    inv_01 = 1.0 / (ALPHA * GAMMA * BETA01)
    for half in range(2):
        lo = half * O_HALF
        out_full[:, :, lo : lo + 2 * OG] *= inv_01
        out_full[:, :, lo + 2 * OG : lo + O_HALF] *= inv_main
    out = np.ascontiguousarray(
        out_full.reshape(B, C, F, H, W).transpose(0, 2, 1, 3, 4)
    )
    return out, res


def kernel(**inputs) -> np.ndarray:
    out, _ = run(inputs, trace=False)
    return out


# revision 15
# speedup vs baseline: 1.0008x; 1.0008x over previous
"""Trainium2 Bass kernel for nn_Channel_dot — hybrid fp8-DoubleRow/bf16.

Math (per batch b):
  x1 = reshape(input1) -> [THW, C];  x2 likewise
  q  = W1 @ x1 + b1            [F, C]
  k  = W2 @ x2 + b2            [F, C]
  sT = k^T q                   [C(d), C(c)]  (sT[d,c] = s[c,d])
  scoresT = softmax over c (free axis of sT)   -- fp32
  out[c,o] = sum_d s[c,d] * (W3 @ x1 + b3)[o,d]
           = sum_i uT[i,c] * W3T[i,o] + r[c]*b3[o]
    where uT[i,c] = sum_d x1[i,d] sT[d,c],  r[c] = sum_d sT[d,c]

Sharding: 8 cores = 4 batches x 2 halves of the G3 output dim (O=16384).

o-groups 0/1 run the direct order (gT = x1^T W3T streamed, then s @ gT)
in pure bf16 — they keep the PE busy while x1/x2 stream in and
q/k/softmax resolve.  o-groups 2..15 use the reassociated order, with
the first 2*P8 i-chunks (of 40) as P8 fp8e4 DoubleRow matmuls (2 chunks
per MM at the same 216ns as one bf16 MM — measured on HW) and the rest
bf16.  P8 is calibrated so global rel-err stays under the 2e-2 gate.

Scale algebra (powers of two, exact): sT is produced ALPHA-scaled, so u
arrives in PSUM as ALPHA*u and casts straight to fp8e4 (clipped +-224
via the two tensor_scalar alu slots); W3 is staged BETA-scaled (fp8
rows e4m3, bf16 rows an exact exponent shift) so fp8 and bf16 chunks
accumulate in ONE psum bank at scale ALPHA*BETA.  og0/1 use GAMMA-
scaled x1; the host divides the output by the per-region scale at the
end.
"""

import os
import sys

for _p in ("/opt/trn_rl_repo", "/root/.axon_site/_ro/trn_rl_repo"):
    if os.path.isdir(_p) and _p not in sys.path:
        sys.path.insert(0, _p)

import numpy as np
import ml_dtypes

import concourse.bacc as bacc
import concourse.bass as bass
import concourse.mybir as mybir
import concourse.tile as tile
from concourse.bass_utils import run_bass_kernel_spmd

B, T, C, H, W = 4, 5, 512, 32, 32
F = 16
THW = T * H * W            # 5120
O_TOT = F * H * W          # 16384
O_HALF = O_TOT // 2        # 8192 per core
NI = THW // 128            # 40 i-chunks
OG = 512                   # o-columns per inner group (1 PSUM bank)
N_OG = O_HALF // OG        # 16
N_MOG = N_OG - 2           # reassociated (main) o-groups
NDT = C // 128             # 4 channel tiles

P8 = 8                     # fp8 pair-chunks per (main og, ct)
NF8 = 2 * P8               # i-chunks covered by fp8
NBF = NI - NF8             # bf16 i-chunks in main o-groups
ALPHA = 2.0 ** 5           # sT scale (alpha*u max ~210 < 240)
BETA = 2.0 ** 13           # W3 scale (beta*W3 max ~115 < 240)
BETA01 = 2.0 ** 8          # W3 scale for og0/1 (g_sb must fit fp16)
GAMMA = 2.0 ** 5           # x1 scale for og0/1 direct path

f32 = mybir.dt.float32
bf16 = mybir.dt.bfloat16
fp16 = mybir.dt.float16
fp8e4 = mybir.dt.float8e4
DR = mybir.MatmulPerfMode.DoubleRow
AF = mybir.ActivationFunctionType
AX = mybir.AxisListType
ALU = mybir.AluOpType
BF16NP = np.dtype(ml_dtypes.bfloat16)
FP16NP = np.dtype(np.float16)
E4M3NP = np.dtype(ml_dtypes.float8_e4m3)

_NC_CACHE = {}


def _chunk_plan(total, step):
    # near-equal parts, each <= step (DMA descriptor batching)
    k = -(-total // step)
    base, rem = divmod(total, k)
    return [base + (i < rem) for i in range(k)]


def _build_nc():
    # Bacc (not plain Bass): its finalize() runs generate_event_semaphores(),
    # which splits multi-wait sync onto EventSemaphore ops — TRN2 compute
    # instructions encode at most one sync wait.
    nc = bacc.Bacc()

    # All streamed inputs are staged partition-major on the host so each
    # DMA reads multi-KB contiguous runs per partition.
    NCH = 5                     # i-chunks per DMA chunk
    x1 = nc.dram_tensor("x1", [128, NI * C], fp16, kind="ExternalInput")
    x2 = nc.dram_tensor("x2", [128, NI * C], fp16, kind="ExternalInput")
    # x1 transposed to channel-major: x1t[p, (n, dt, j)] = x1[i=n*128+j,
    # c=dt*128+p] — the u-phase contracts over the channel dim.
    x1t = nc.dram_tensor("x1t", [128, NI * NDT * 128], fp16, kind="ExternalInput")
    w1t = nc.dram_tensor("w1t", [128, NI * F], fp16, kind="ExternalInput")
    w2t = nc.dram_tensor("w2t", [128, NI * F], fp16, kind="ExternalInput")
    # W3 streams: og0/1 full-bf16; main ogs split fp8-pairs + bf16 tail.
    w3g = nc.dram_tensor("w3g", [128, 2 * NI * OG], fp16, kind="ExternalInput")
    w3q = nc.dram_tensor("w3q", [128, N_MOG * P8 * 2 * OG], fp8e4,
                         kind="ExternalInput")
    w3b = nc.dram_tensor("w3b", [128, N_MOG * NBF * OG], fp16,
                         kind="ExternalInput")
    b1 = nc.dram_tensor("b1", [F, 1], f32, kind="ExternalInput")
    b2 = nc.dram_tensor("b2", [F, 1], f32, kind="ExternalInput")
    # b3 replicated to 128 partitions on the host; streamed per o-group.
    b3b = nc.dram_tensor("b3b", [128, O_HALF], bf16, kind="ExternalInput")
    b3g = nc.dram_tensor("b3g", [128, 2 * OG], fp16, kind="ExternalInput")
    out = nc.dram_tensor("out", [C, O_HALF], f32, kind="ExternalOutput")

    x1_r = x1.rearrange("p (n c) -> p n c", c=C)
    x2_r = x2.rearrange("p (n c) -> p n c", c=C)
    x1t_r = x1t.rearrange("p (n dt j) -> p n dt j", dt=NDT, j=128)
    w3g_r = w3g.rearrange("p (og n oc) -> p og n oc", og=2, n=NI)
    w3q_r = w3q.rearrange("p (og j s oc) -> p og j s oc", og=N_MOG, j=P8, s=2)
    w3b_r = w3b.rearrange("p (og m oc) -> p og m oc", og=N_MOG, m=NBF)
    w1_r = w1t.rearrange("p (n f) -> p n f", f=F)
    w2_r = w2t.rearrange("p (n f) -> p n f", f=F)
    out_r = out.rearrange("(ct p) o -> ct p o", p=128)

    with tile.TileContext(nc) as tc:
        with (
            tc.tile_pool(name="persist", bufs=1) as persist,
            tc.tile_pool(name="w3p", bufs=7) as w3p,
            tc.tile_pool(name="w3qp", bufs=3) as w3qp,
            tc.tile_pool(name="gsbp", bufs=2) as gsbp,
            tc.tile_pool(name="outp", bufs=4) as outp,
            tc.tile_pool(name="b3p", bufs=3) as b3p,
            tc.tile_pool(name="tmpp", bufs=8) as tmpp,
            tc.tile_pool(name="small", bufs=2) as small,
            tc.tile_pool(name="pg", bufs=5, space="PSUM") as pg,
            tc.tile_pool(name="po", bufs=2, space="PSUM") as po,
            tc.tile_pool(name="pqk", bufs=1, space="PSUM") as pqk,
        ):
            # ---- persistent tiles ----
            # x1 (i-major) is dead after og0/1's g-streams + q; the
            # x1T layout reuses its SBUF slot via the shared tag.
            x1_sb = persist.tile([128, NI, C], fp16, name="x1_sb", tag="x1x")
            x1t_sb = persist.tile(
                [128, NI, NDT, 128], fp16, name="x1t_sb", tag="x1x"
            )
            # x2 is dead after k; uT (bf16 + fp8 parts) reuses its slot.
            x2_sb = persist.tile([128, NI, C], fp16, name="x2_sb", tag="xu")
            ut_sb = persist.tile([128, NBF, C], fp16, name="ut_sb", tag="xu")
            ut8_sb = persist.tile([128, P8, 2, C], fp8e4, name="ut8_sb")
            sT_sb = persist.tile([128, NDT, C], fp16, name="sT_sb")

            def b3_tile(og):
                dt_b3 = fp16 if og < 2 else bf16
                b3_t = b3p.tile([128, OG], dt_b3, name="b3_t")
                src_t = b3g if og < 2 else b3b
                nc.sync.dma_start(out=b3_t[:], in_=src_t[:, og * OG : (og + 1) * OG])
                return b3_t

            def w3q_fetch(og):
                w3q_t = w3qp.tile([128, P8, 2, OG], fp8e4, name="w3q_t")
                nc.sync.dma_start(out=w3q_t[:], in_=w3q_r[:, og - 2])
                return w3q_t

            def g_phase(og, x_load=None):
                """Direct-order o-group (0/1): stream W3 columns, accumulate
                gT = (GAMMA x1)^T (BETA W3T) in PSUM, pure bf16."""
                g_ps_l = [pg.tile([128, OG], f32, name="g_ps") for _ in range(NDT)]
                # og 0 ramps with fine-grained chunks so the very first
                # matmul starts as early as possible (DMA queues are still
                # spinning up during the first ~15us)
                plan = [1, 1, 2, 3, 4, 4, 5, 5, 5, 5, 5] if og == 0 else \
                    _chunk_plan(NI, NCH)
                n0 = 0
                for ch in plan:
                    if x_load is not None:
                        # one x chunk rides along per w3 chunk so the
                        # prologue inputs arrive without their own phase
                        nc.sync.dma_start(
                            out=x_load[0][:, n0 : n0 + ch, :],
                            in_=x_load[1][:, n0 : n0 + ch, :],
                        )
                    w3_t = w3p.tile([128, NCH, OG], fp16, name="w3_t")
                    nc.sync.dma_start(
                        out=w3_t[:, :ch, :], in_=w3g_r[:, og, n0 : n0 + ch, :]
                    )
                    for j in range(ch):
                        for dt_ in range(NDT):
                            nc.tensor.matmul(
                                g_ps_l[dt_][:],
                                lhsT=x1_sb[:, n0 + j, dt_ * 128 : (dt_ + 1) * 128],
                                rhs=w3_t[:, j, :],
                                start=(n0 + j == 0),
                                stop=(n0 + j == NI - 1),
                            )
                    n0 += ch
                return g_ps_l

            def evac_phase(g_ps_l, b3_t):
                """Evacuate gT (+b3) to SBUF right after its g-stream ends,
                while the Vector engine is idle."""
                g_sb = gsbp.tile([128, NDT, OG], fp16, name="g_sb")
                for dt_ in range(NDT):
                    nc.vector.tensor_add(
                        g_sb[:, dt_, :], g_ps_l[dt_][:], b3_t[:]
                    )
                return g_sb

            def out_phase(og, g_sb):
                """scores @ gT for a direct-order o-group."""
                osl = slice(og * OG, (og + 1) * OG)
                for ct in range(NDT):
                    o_ps = po.tile([128, OG], f32, name="o_ps", tag="so")
                    for dt_ in range(NDT):
                        nc.tensor.matmul(
                            o_ps[:],
                            lhsT=sT_sb[:, dt_, ct * 128 : (ct + 1) * 128],
                            rhs=g_sb[:, dt_, :],
                            start=(dt_ == 0),
                            stop=(dt_ == NDT - 1),
                        )
                    out_t = outp.tile([128, OG], f32, name="out_t")
                    nc.vector.tensor_copy(out_t[:], o_ps[:])
                    nc.sync.dma_start(out=out_r[ct, :, osl], in_=out_t[:])

            def bog_begin(og):
                """First half of a reassociated o-group: bias prep + the P8
                fp8-DoubleRow MMs (only need ut8 + the prefetched w3q)."""
                b3_t = b3_tile(og)
                # rank-1 bias term precomputed on DVE; consumed by the
                # PSUM evacuation adds at the end of the stream
                tmp_l = []
                for ct in range(NDT):
                    tmp_t = tmpp.tile([128, OG], bf16, name="tmp_t")
                    nc.vector.tensor_scalar_mul(
                        tmp_t[:], b3_t[:], r_sb[:, ct : ct + 1]
                    )
                    tmp_l.append(tmp_t)
                ps_l = [pg.tile([128, OG], f32, name="g_ps") for _ in range(NDT)]
                w3q_t = w3q_pending.pop(og, None)
                if w3q_t is None:
                    w3q_t = w3q_fetch(og)
                for j in range(P8):
                    for ct in range(NDT):
                        nc.tensor.matmul(
                            ps_l[ct][:],
                            lhsT=ut8_sb[:, j, :, ct * 128 : (ct + 1) * 128],
                            rhs=w3q_t[:, j, :, :],
                            start=(j == 0),
                            stop=False,
                            perf_mode=DR,
                        )
                return ps_l, tmp_l

            def bog_finish(og, ps_l, tmp_l):
                """Second half: the NBF fp16 MMs + evacuation.  The last
                chunk group runs ct-major with per-ct stop + immediate
                evacuation, so three of the four PSUM banks free while the
                tail MMs still stream — the next o-group's first matmuls
                then never wait on this one's evacuation DVE ops."""
                osl = slice(og * OG, (og + 1) * OG)
                mog = og - 2
                plan = _chunk_plan(NBF, NCH)
                m0 = 0
                for gi, ch in enumerate(plan):
                    w3_t = w3p.tile([128, NCH, OG], fp16, name="w3_t")
                    nc.sync.dma_start(
                        out=w3_t[:, :ch, :], in_=w3b_r[:, mog, m0 : m0 + ch, :]
                    )
                    if gi < len(plan) - 1:
                        for j in range(ch):
                            for ct in range(NDT):
                                nc.tensor.matmul(
                                    ps_l[ct][:],
                                    lhsT=ut_sb[:, m0 + j, ct * 128 : (ct + 1) * 128],
                                    rhs=w3_t[:, j, :],
                                    start=False,
                                    stop=False,
                                )
                    else:
                        for ct in range(NDT):
                            for j in range(ch):
                                nc.tensor.matmul(
                                    ps_l[ct][:],
                                    lhsT=ut_sb[:, m0 + j, ct * 128 : (ct + 1) * 128],
                                    rhs=w3_t[:, j, :],
                                    start=False,
                                    stop=(j == ch - 1),
                                )
                            out_t = outp.tile([128, OG], f32, name="out_t")
                            nc.vector.tensor_add(
                                out_t[:], ps_l[ct][:], tmp_l[ct][:]
                            )
                            nc.sync.dma_start(out=out_r[ct, :, osl], in_=out_t[:])
                    m0 += ch

            def bog_phase_ct_major(og):
                """Last o-group runs ct-major: each row tile's accumulation
                completes before the next starts, so its evacuation + out
                DMA overlap the remaining tiles' matmuls (shorter tail)."""
                osl = slice(og * OG, (og + 1) * OG)
                mog = og - 2
                b3_t = b3_tile(og)
                tmp_l = []
                for ct in range(NDT):
                    tmp_t = tmpp.tile([128, OG], bf16, name="tmp_t")
                    nc.vector.tensor_scalar_mul(
                        tmp_t[:], b3_t[:], r_sb[:, ct : ct + 1]
                    )
                    tmp_l.append(tmp_t)
                ps_l = [pg.tile([128, OG], f32, name="g_ps") for _ in range(NDT)]
                w3q_t = w3q_pending.pop(og, None)
                if w3q_t is None:
                    w3q_t = w3q_fetch(og)
                w3_tl = []
                m0 = 0
                for ch in _chunk_plan(NBF, NCH):
                    w3_t = w3p.tile([128, NCH, OG], fp16, name="w3_t")
                    nc.sync.dma_start(
                        out=w3_t[:, :ch, :], in_=w3b_r[:, mog, m0 : m0 + ch, :]
                    )
                    w3_tl.append((w3_t, m0, ch))
                    m0 += ch
                for ct in range(NDT):
                    for j in range(P8):
                        nc.tensor.matmul(
                            ps_l[ct][:],
                            lhsT=ut8_sb[:, j, :, ct * 128 : (ct + 1) * 128],
                            rhs=w3q_t[:, j, :, :],
                            start=(j == 0),
                            stop=False,
                            perf_mode=DR,
                        )
                    for w3_t, m0, ch in w3_tl:
                        for j in range(ch):
                            nc.tensor.matmul(
                                ps_l[ct][:],
                                lhsT=ut_sb[:, m0 + j, ct * 128 : (ct + 1) * 128],
                                rhs=w3_t[:, j, :],
                                start=False,
                                stop=(m0 + j == NBF - 1),
                            )
                    out_t = outp.tile([128, OG], f32, name="out_t")
                    nc.vector.tensor_add(out_t[:], ps_l[ct][:], tmp_l[ct][:])
                    nc.sync.dma_start(out=out_r[ct, :, osl], in_=out_t[:])

            # o-group 0's g-stream first, with x1 loads interleaved: the PE
            # starts as soon as the first x1/W3 tile pair lands.
            g0 = g_phase(0, x_load=(x1_sb, x1_r))  # x1 rides og0's stream
            b3_t0 = b3_tile(0)

            # W1T/W2T zero-padded on-chip to 128 output columns: M=128
            # matmuls get fast weight load while only 160KB each moves.
            w1t_sb = persist.tile([128, NI, 128], fp16, name="w1t_sb")
            nc.vector.memset(w1t_sb[:], 0.0)
            nc.sync.dma_start(out=w1t_sb[:, :, :F], in_=w1_r[:])
            w2t_sb = persist.tile([128, NI, 128], fp16, name="w2t_sb")
            nc.vector.memset(w2t_sb[:], 0.0)
            nc.sync.dma_start(out=w2t_sb[:, :, :F], in_=w2_r[:])
            b1_sb = persist.tile([F, 1], f32, name="b1_sb")
            nc.sync.dma_start(out=b1_sb[:], in_=b1[:])
            b2_sb = persist.tile([F, 1], f32, name="b2_sb")
            nc.sync.dma_start(out=b2_sb[:], in_=b2[:])
            ones_sb = persist.tile([128, 1], fp16, name="ones_sb")
            nc.vector.memset(ones_sb[:], 1.0)

            # ---- q = (W1/GAMMA) @ (GAMMA x1) + b1 -> [F, C] fp32 ----
            q_ps = pqk.tile([128, C], f32, name="q_ps", tag="qk")
            for n in range(NI):
                nc.tensor.matmul(
                    q_ps[:],
                    lhsT=w1t_sb[:, n, :],
                    rhs=x1_sb[:, n, :],
                    start=(n == 0),
                    stop=(n == NI - 1),
                )
            q_sb = persist.tile([F, C], f32, name="q_sb")
            nc.vector.tensor_scalar_add(q_sb[:], q_ps[:F, :], b1_sb[:])

            # og0's gT evacuates now (Vector is idle; g0 psum is complete)
            g_sb0 = evac_phase(g0, b3_t0)

            # o-group 1's g-stream carries the x2 loads (k runs after it)
            g1 = g_phase(1, x_load=(x2_sb, x2_r))
            b3_t1 = b3_tile(1)

            # ---- k = W2 @ x2 + b2 -> [F, C] fp32 ----
            k_ps = pqk.tile([128, C], f32, name="k_ps", tag="qk")
            for n in range(NI):
                nc.tensor.matmul(
                    k_ps[:],
                    lhsT=w2t_sb[:, n, :],
                    rhs=x2_sb[:, n, :],
                    start=(n == 0),
                    stop=(n == NI - 1),
                )
            k_sb = persist.tile([F, C], f32, name="k_sb")
            nc.vector.tensor_scalar_add(k_sb[:], k_ps[:F, :], b2_sb[:])

            # og1's gT evacuates immediately too
            g_sb1 = evac_phase(g1, b3_t1)

            # x1T streams in while softmax/out-phases run; the u-phase
            # consumes it granule by granule.  og2/og3's fp8 W3 streams
            # prefetch here (interleaved) so og2's DoubleRow matmuls don't
            # stall on DMA right after the u-phase.
            w3q_pending = {}
            for gch in range(NI // NCH):
                nc.sync.dma_start(
                    out=x1t_sb[:, gch * NCH : (gch + 1) * NCH, :, :],
                    in_=x1t_r[:, gch * NCH : (gch + 1) * NCH, :, :],
                )
                if gch == 0:
                    w3q_pending[2] = w3q_fetch(2)
                elif gch == 1:
                    w3q_pending[3] = w3q_fetch(3)

            # ---- sT[d, c] = sum_f k[f,d] q[f,c] (plain fp32 matmul),
            #      then softmax over free (c); emit ALPHA-scaled bf16
            #      scores.  The tiny r-matmuls (r[c] = sum_d sT[d,c],
            #      partition reduce via a ones vector) interleave into the
            #      softmax window. ----
            # four separate PSUM tiles: a column-sliced accumulation in one
            # bank corrupts sibling columns (start=True resets the bank).
            r_ps_l = [pg.tile([128, 1], f32, name="g_ps") for _ in range(NDT)]

            def softmax_tail(dt_, s_ps):
                # logits are bounded (|s| < ~10 for this problem), so plain
                # exp is fp32-safe; skipping the max keeps Exp at one sync
                # wait (the Activation ISA slot allows only one).
                e_sb = small.tile([128, C], f32, name="e_sb")
                esum = small.tile([128, 1], f32, name="esum")
                nc.scalar.activation(
                    e_sb[:], s_ps[:], AF.Exp, scale=1.0, accum_out=esum[:],
                )
                rcp = small.tile([128, 1], f32, name="rcp")
                nc.vector.reciprocal(rcp[:], esum[:])
                rcp_a = small.tile([128, 1], f32, name="rcp_a")
                nc.vector.tensor_scalar_mul(rcp_a[:], rcp[:], float(ALPHA))
                nc.vector.tensor_scalar_mul(sT_sb[:, dt_, :], e_sb[:], rcp_a[:])
                for ct in range(NDT):
                    nc.tensor.matmul(
                        r_ps_l[ct][:],
                        lhsT=sT_sb[:, dt_, ct * 128 : (ct + 1) * 128],
                        rhs=ones_sb[:],
                        start=(dt_ == 0),
                        stop=(dt_ == NDT - 1),
                    )

            s_pend = None
            for dt_ in range(NDT):
                s_ps = po.tile([128, C], f32, name="s_ps", tag="so")
                nc.tensor.matmul(
                    s_ps[:],
                    lhsT=k_sb[:, dt_ * 128 : (dt_ + 1) * 128],
                    rhs=q_sb[:],
                    start=True,
                    stop=True,
                )
                if s_pend is not None:
                    softmax_tail(dt_ - 1, s_pend)
                s_pend = s_ps
            softmax_tail(NDT - 1, s_pend)
            r_sb = persist.tile([128, NDT], f32, name="r_sb")
            for ct in range(NDT):
                nc.vector.tensor_copy(r_sb[:, ct : ct + 1], r_ps_l[ct][:])

            # ---- direct-order output for o-groups 0/1 ----
            out_phase(0, g_sb0)
            out_phase(1, g_sb1)

            # ---- u-phase: uT[i, c] = ALPHA * sum_d x1[i,d] s[c,d] ----
            # first NF8 chunks quantize to fp8e4 (clipped +-224 via the two
            # tensor_scalar alu slots); the rest evacuate fp16.
            def u_chunk(n):
                u_ps = po.tile([128, C], f32, name="u_ps", tag="so")
                for dt_ in range(NDT):
                    nc.tensor.matmul(
                        u_ps[:],
                        lhsT=x1t_sb[:, n, dt_, :],
                        rhs=sT_sb[:, dt_, :],
                        start=(dt_ == 0),
                        stop=(dt_ == NDT - 1),
                    )
                if n < NF8:
                    nc.vector.tensor_scalar(
                        ut8_sb[:, n // 2, n % 2, :], u_ps[:],
                        224.0, -224.0, ALU.min, ALU.max,
                    )
                else:
                    nc.vector.tensor_copy(ut_sb[:, n - NF8, :], u_ps[:])

            for n in range(NF8):
                u_chunk(n)
            # og2's fp8-DoubleRow block interleaves here: its inputs (ut8 +
            # the prefetched w3q) are ready, and it gives the x1t stream
            # breathing room so the tail u-chunks don't stall on DMA.
            og2_ps, og2_tmp = bog_begin(2)
            # The tail u-chunks and og2's fp16 stream merge, staggered by
            # one chunk (og2's block m needs ut[m], evacuated one round
            # earlier): halves the x1t consumption rate so its DMA stream
            # keeps pace, and keeps the PE dense through this window.
            u_chunk(NF8)
            osl2 = slice(2 * OG, 3 * OG)
            plan2 = _chunk_plan(NBF, NCH)
            gi = 0
            j_in = 0
            ch2 = plan2[0]
            w3_t2 = None
            for m in range(NBF):
                if j_in == 0:
                    ch2 = plan2[gi]
                    w3_t2 = w3p.tile([128, NCH, OG], fp16, name="w3_t")
                    nc.sync.dma_start(
                        out=w3_t2[:, :ch2, :], in_=w3b_r[:, 0, m : m + ch2, :]
                    )
                if NF8 + m + 1 < NI:
                    u_chunk(NF8 + m + 1)
                for ct in range(NDT):
                    nc.tensor.matmul(
                        og2_ps[ct][:],
                        lhsT=ut_sb[:, m, ct * 128 : (ct + 1) * 128],
                        rhs=w3_t2[:, j_in, :],
                        start=False,
                        stop=(m == NBF - 1),
                    )
                j_in += 1
                if j_in == ch2:
                    j_in = 0
                    gi += 1
            for ct in range(NDT):
                out_t = outp.tile([128, OG], f32, name="out_t")
                nc.vector.tensor_add(out_t[:], og2_ps[ct][:], og2_tmp[ct][:])
                nc.sync.dma_start(out=out_r[ct, :, osl2], in_=out_t[:])

            # ---- main: reassociated hybrid stream for o-groups 3..15 ----
            for og in range(3, N_OG - 1):
                ps_l, tmp_l = bog_begin(og)
                bog_finish(og, ps_l, tmp_l)
            bog_phase_ct_major(N_OG - 1)

    nc.finalize()
    return nc


def _get_nc():
    if "nc" not in _NC_CACHE:
        _NC_CACHE["nc"] = _build_nc()
    return _NC_CACHE["nc"]


def _stage_inputs(input1, input2, W1, b1, W2, b2, W3, b3):
    input1 = np.asarray(input1, np.float32)
    input2 = np.asarray(input2, np.float32)
    W1 = np.asarray(W1, np.float32)
    W2 = np.asarray(W2, np.float32)
    W3 = np.asarray(W3, np.float32)
    b1 = np.asarray(b1, np.float32)
    b2 = np.asarray(b2, np.float32)
    b3 = np.asarray(b3, np.float32)

    def pmajor(X, inner):
        # [THW, inner] -> [128, NI*inner]: row p = concat_n X[n*128+p, :]
        return np.ascontiguousarray(
            X.reshape(-1, 128, inner).transpose(1, 0, 2).reshape(128, -1)
        )

    # [B,T,C,H,W] -> x[b][i=(t,hw), c], partition-major
    X1f = np.ascontiguousarray(
        input1.reshape(B, T, C, H * W).transpose(0, 1, 3, 2)
    ).reshape(B, THW, C)
    X2f = np.ascontiguousarray(
        input2.reshape(B, T, C, H * W).transpose(0, 1, 3, 2)
    ).reshape(B, THW, C)
    X1g = (GAMMA * X1f).astype(FP16NP)          # gamma-scaled fp16
    X2 = X2f.astype(FP16NP)
    X1p = [pmajor(X1g[b], C) for b in range(B)]
    X2p = [pmajor(X2[b], C) for b in range(B)]
    # channel-major x1 (unscaled): [128p, (n, dt, j)] = x1[n*128+j, dt*128+p]
    X1b = X1f.astype(FP16NP)
    X1Tp = [
        np.ascontiguousarray(
            X1b[b].reshape(NI, 128, NDT, 128).transpose(3, 0, 2, 1)
        ).reshape(128, NI * NDT * 128)
        for b in range(B)
    ]
    W1Tp = pmajor(np.ascontiguousarray((W1 / GAMMA).T).astype(FP16NP), F)
    W2Tp = pmajor(np.ascontiguousarray(W2.T).astype(FP16NP), F)
    W3T = np.ascontiguousarray(W3.T)             # [THW, O_TOT]
    W3q8 = (BETA * W3T[: NF8 * 128]).astype(E4M3NP)   # fp8 rows (main ogs)
    W3bb = (BETA * W3T).astype(FP16NP)           # fp16 rows, main ogs
    W3gg = (BETA01 * W3T).astype(FP16NP)         # fp16 rows, og0/1
    b1c = np.ascontiguousarray(b1.reshape(F, 1))
    b2c = np.ascontiguousarray(b2.reshape(F, 1))

    in_maps = []
    for core in range(8):
        b = core // 2
        half = core % 2
        osl = slice(half * O_HALF, (half + 1) * O_HALF)
        W3h8 = W3q8[:, osl]
        W3hb = W3bb[:, osl]
        W3hg = W3gg[:, osl]
        # og0/1: full-depth fp16 [128, (og2, n40, oc)]
        w3g_core = np.ascontiguousarray(
            W3hg[:, : 2 * OG]
            .reshape(NI, 128, 2, OG)
            .transpose(1, 2, 0, 3)
            .reshape(128, 2 * NI * OG)
        )
        # main ogs fp8 pairs -> [128, (og, j, s, oc)]
        w3q_core = np.ascontiguousarray(
            W3h8[:, 2 * OG :]
            .reshape(P8, 2, 128, N_MOG, OG)
            .transpose(2, 3, 0, 1, 4)
            .reshape(128, N_MOG * P8 * 2 * OG)
        )
        # main ogs bf16 tail -> [128, (og, m, oc)]
        w3b_core = np.ascontiguousarray(
            W3hb[NF8 * 128 :, 2 * OG :]
            .reshape(NBF, 128, N_MOG, OG)
            .transpose(1, 2, 0, 3)
            .reshape(128, N_MOG * NBF * OG)
        )
        b3h = b3[osl]
        in_maps.append(
            {
                "x1": X1p[b],
                "x2": X2p[b],
                "x1t": X1Tp[b],
                "w1t": W1Tp,
                "w2t": W2Tp,
                "w3g": w3g_core,
                "w3q": w3q_core,
                "w3b": w3b_core,
                "b1": b1c,
                "b2": b2c,
                "b3b": np.ascontiguousarray(
                    np.broadcast_to(
                        (BETA * b3h).astype(BF16NP)[None, :], (128, O_HALF)
                    )
                ),
                "b3g": np.ascontiguousarray(
                    np.broadcast_to(
                        (GAMMA * BETA01 * b3h[: 2 * OG]).astype(FP16NP)[None, :],
                        (128, 2 * OG),
                    )
                ),
            }
        )
    return in_maps


def run(inputs: dict, trace: bool = False):
    """Returns (full_output [B,F,C,H,W], BassKernelResults)."""
    in_maps = _stage_inputs(**inputs)
    nc = _get_nc()
    res = run_bass_kernel_spmd(nc, in_maps, core_ids=list(range(8)), trace=trace)
    out_full = np.empty((B, C, O_TOT), np.float32)
    for core in range(8):
        b = core // 2
        half = core % 2
        out_full[b, :, half * O_HALF : (half + 1) * O_HALF] = res.results[core]["out"]
    # host unscale: og0/1 of each half at ALPHA*GAMMA*BETA, rest ALPHA*BETA
    inv_main = 1.0 / (ALPHA * BETA)
    inv_01 = 1.0 / (ALPHA * GAMMA * BETA01)
    for half in range(2):
        lo = half * O_HALF
        out_full[:, :, lo : lo + 2 * OG] *= inv_01
        out_full[:, :, lo + 2 * OG : lo + O_HALF] *= inv_main
    out = np.ascontiguousarray(
        out_full.reshape(B, C, F, H, W).transpose(0, 2, 1, 3, 4)
    )
    return out, res


def kernel(**inputs) -> np.ndarray:
    out, _ = run(inputs, trace=False)
    return out


# revision 17
# speedup vs baseline: 1.0037x; 1.0030x over previous
"""Trainium2 Bass kernel for nn_Channel_dot — hybrid fp8-DoubleRow/bf16.

Math (per batch b):
  x1 = reshape(input1) -> [THW, C];  x2 likewise
  q  = W1 @ x1 + b1            [F, C]
  k  = W2 @ x2 + b2            [F, C]
  sT = k^T q                   [C(d), C(c)]  (sT[d,c] = s[c,d])
  scoresT = softmax over c (free axis of sT)   -- fp32
  out[c,o] = sum_d s[c,d] * (W3 @ x1 + b3)[o,d]
           = sum_i uT[i,c] * W3T[i,o] + r[c]*b3[o]
    where uT[i,c] = sum_d x1[i,d] sT[d,c],  r[c] = sum_d sT[d,c]

Sharding: 8 cores = 4 batches x 2 halves of the G3 output dim (O=16384).

o-groups 0/1 run the direct order (gT = x1^T W3T streamed, then s @ gT)
in pure bf16 — they keep the PE busy while x1/x2 stream in and
q/k/softmax resolve.  o-groups 2..15 use the reassociated order, with
the first 2*P8 i-chunks (of 40) as P8 fp8e4 DoubleRow matmuls (2 chunks
per MM at the same 216ns as one bf16 MM — measured on HW) and the rest
bf16.  P8 is calibrated so global rel-err stays under the 2e-2 gate.

Scale algebra (powers of two, exact): sT is produced ALPHA-scaled, so u
arrives in PSUM as ALPHA*u and casts straight to fp8e4 (clipped +-224
via the two tensor_scalar alu slots); W3 is staged BETA-scaled (fp8
rows e4m3, bf16 rows an exact exponent shift) so fp8 and bf16 chunks
accumulate in ONE psum bank at scale ALPHA*BETA.  og0/1 use GAMMA-
scaled x1; the host divides the output by the per-region scale at the
end.
"""

import os
import sys

for _p in ("/opt/trn_rl_repo", "/root/.axon_site/_ro/trn_rl_repo"):
    if os.path.isdir(_p) and _p not in sys.path:
        sys.path.insert(0, _p)

import numpy as np
import ml_dtypes

import concourse.bacc as bacc
import concourse.bass as bass
import concourse.mybir as mybir
import concourse.tile as tile
from concourse.bass_utils import run_bass_kernel_spmd

B, T, C, H, W = 4, 5, 512, 32, 32
F = 16
THW = T * H * W            # 5120
O_TOT = F * H * W          # 16384
O_HALF = O_TOT // 2        # 8192 per core
NI = THW // 128            # 40 i-chunks
OG = 512                   # o-columns per inner group (1 PSUM bank)
N_OG = O_HALF // OG        # 16
N_MOG = N_OG - 2           # reassociated (main) o-groups
NDT = C // 128             # 4 channel tiles

P8 = 8                     # fp8 pair-chunks per (main og, ct)
NF8 = 2 * P8               # i-chunks covered by fp8
NBF = NI - NF8             # bf16 i-chunks in main o-groups
ALPHA = 2.0 ** 5           # sT scale (alpha*u max ~210 < 240)
BETA = 2.0 ** 13           # W3 scale (beta*W3 max ~115 < 240)
BETA01 = 2.0 ** 8          # W3 scale for og0/1 (g_sb must fit fp16)
GAMMA = 2.0 ** 5           # x1 scale for og0/1 direct path

f32 = mybir.dt.float32
bf16 = mybir.dt.bfloat16
fp16 = mybir.dt.float16
fp8e4 = mybir.dt.float8e4
DR = mybir.MatmulPerfMode.DoubleRow
AF = mybir.ActivationFunctionType
AX = mybir.AxisListType
ALU = mybir.AluOpType
BF16NP = np.dtype(ml_dtypes.bfloat16)
FP16NP = np.dtype(np.float16)
E4M3NP = np.dtype(ml_dtypes.float8_e4m3)

_NC_CACHE = {}


def _chunk_plan(total, step):
    # near-equal parts, each <= step (DMA descriptor batching)
    k = -(-total // step)
    base, rem = divmod(total, k)
    return [base + (i < rem) for i in range(k)]


def _build_nc():
    # Bacc (not plain Bass): its finalize() runs generate_event_semaphores(),
    # which splits multi-wait sync onto EventSemaphore ops — TRN2 compute
    # instructions encode at most one sync wait.
    nc = bacc.Bacc()

    # All streamed inputs are staged partition-major on the host so each
    # DMA reads multi-KB contiguous runs per partition.
    NCH = 5                     # i-chunks per DMA chunk
    x1 = nc.dram_tensor("x1", [128, NI * C], fp16, kind="ExternalInput")
    x2 = nc.dram_tensor("x2", [128, NI * C], fp16, kind="ExternalInput")
    # x1 transposed to channel-major: x1t[p, (n, dt, j)] = x1[i=n*128+j,
    # c=dt*128+p] — the u-phase contracts over the channel dim.
    x1t = nc.dram_tensor("x1t", [128, NI * NDT * 128], fp16, kind="ExternalInput")
    w1t = nc.dram_tensor("w1t", [128, NI * F], fp16, kind="ExternalInput")
    w2t = nc.dram_tensor("w2t", [128, NI * F], fp16, kind="ExternalInput")
    # W3 streams: og0/1 full-bf16; main ogs split fp8-pairs + bf16 tail.
    w3g = nc.dram_tensor("w3g", [128, 2 * NI * OG], fp16, kind="ExternalInput")
    w3q = nc.dram_tensor("w3q", [128, N_MOG * P8 * 2 * OG], fp8e4,
                         kind="ExternalInput")
    w3b = nc.dram_tensor("w3b", [128, N_MOG * NBF * OG], fp16,
                         kind="ExternalInput")
    b1 = nc.dram_tensor("b1", [F, 1], f32, kind="ExternalInput")
    b2 = nc.dram_tensor("b2", [F, 1], f32, kind="ExternalInput")
    # b3 replicated to 128 partitions on the host; streamed per o-group.
    b3b = nc.dram_tensor("b3b", [128, O_HALF], bf16, kind="ExternalInput")
    b3g = nc.dram_tensor("b3g", [128, 2 * OG], fp16, kind="ExternalInput")
    out = nc.dram_tensor("out", [C, O_HALF], f32, kind="ExternalOutput")

    x1_r = x1.rearrange("p (n c) -> p n c", c=C)
    x2_r = x2.rearrange("p (n c) -> p n c", c=C)
    x1t_r = x1t.rearrange("p (n dt j) -> p n dt j", dt=NDT, j=128)
    w3g_r = w3g.rearrange("p (og n oc) -> p og n oc", og=2, n=NI)
    w3q_r = w3q.rearrange("p (og j s oc) -> p og j s oc", og=N_MOG, j=P8, s=2)
    w3b_r = w3b.rearrange("p (og m oc) -> p og m oc", og=N_MOG, m=NBF)
    w1_r = w1t.rearrange("p (n f) -> p n f", f=F)
    w2_r = w2t.rearrange("p (n f) -> p n f", f=F)
    out_r = out.rearrange("(ct p) o -> ct p o", p=128)

    with tile.TileContext(nc) as tc:
        with (
            tc.tile_pool(name="persist", bufs=1) as persist,
            tc.tile_pool(name="w3p", bufs=7) as w3p,
            tc.tile_pool(name="w3qp", bufs=3) as w3qp,
            tc.tile_pool(name="gsbp", bufs=2) as gsbp,
            tc.tile_pool(name="outp", bufs=4) as outp,
            tc.tile_pool(name="b3p", bufs=3) as b3p,
            tc.tile_pool(name="tmpp", bufs=8) as tmpp,
            tc.tile_pool(name="small", bufs=2) as small,
            tc.tile_pool(name="pg", bufs=5, space="PSUM") as pg,
            tc.tile_pool(name="po", bufs=2, space="PSUM") as po,
            tc.tile_pool(name="pqk", bufs=1, space="PSUM") as pqk,
        ):
            # ---- persistent tiles ----
            # x1 (i-major) is dead after og0/1's g-streams + q; the
            # x1T layout reuses its SBUF slot via the shared tag.
            x1_sb = persist.tile([128, NI, C], fp16, name="x1_sb", tag="x1x")
            x1t_sb = persist.tile(
                [128, NI, NDT, 128], fp16, name="x1t_sb", tag="x1x"
            )
            # x2 is dead after k; uT (bf16 + fp8 parts) reuses its slot.
            x2_sb = persist.tile([128, NI, C], fp16, name="x2_sb", tag="xu")
            ut_sb = persist.tile([128, NBF, C], fp16, name="ut_sb", tag="xu")
            ut8_sb = persist.tile([128, P8, 2, C], fp8e4, name="ut8_sb")
            sT_sb = persist.tile([128, NDT, C], fp16, name="sT_sb")

            def b3_tile(og):
                dt_b3 = fp16 if og < 2 else bf16
                b3_t = b3p.tile([128, OG], dt_b3, name="b3_t")
                src_t = b3g if og < 2 else b3b
                nc.sync.dma_start(out=b3_t[:], in_=src_t[:, og * OG : (og + 1) * OG])
                return b3_t

            def w3q_fetch(og):
                w3q_t = w3qp.tile([128, P8, 2, OG], fp8e4, name="w3q_t")
                nc.sync.dma_start(out=w3q_t[:], in_=w3q_r[:, og - 2])
                return w3q_t

            def g_phase(og, x_load=None):
                """Direct-order o-group (0/1): stream W3 columns, accumulate
                gT = (GAMMA x1)^T (BETA W3T) in PSUM, pure bf16."""
                g_ps_l = [pg.tile([128, OG], f32, name="g_ps") for _ in range(NDT)]
                # og 0 ramps with fine-grained chunks so the very first
                # matmul starts as early as possible (DMA queues are still
                # spinning up during the first ~15us)
                plan = [1, 1, 2, 3, 4, 4, 5, 5, 5, 5, 5] if og == 0 else \
                    _chunk_plan(NI, NCH)
                n0 = 0
                for ch in plan:
                    if x_load is not None:
                        # one x chunk rides along per w3 chunk so the
                        # prologue inputs arrive without their own phase
                        nc.sync.dma_start(
                            out=x_load[0][:, n0 : n0 + ch, :],
                            in_=x_load[1][:, n0 : n0 + ch, :],
                        )
                    w3_t = w3p.tile([128, NCH, OG], fp16, name="w3_t")
                    nc.sync.dma_start(
                        out=w3_t[:, :ch, :], in_=w3g_r[:, og, n0 : n0 + ch, :]
                    )
                    for j in range(ch):
                        for dt_ in range(NDT):
                            nc.tensor.matmul(
                                g_ps_l[dt_][:],
                                lhsT=x1_sb[:, n0 + j, dt_ * 128 : (dt_ + 1) * 128],
                                rhs=w3_t[:, j, :],
                                start=(n0 + j == 0),
                                stop=(n0 + j == NI - 1),
                            )
                    n0 += ch
                return g_ps_l

            def evac_phase(g_ps_l, b3_t):
                """Evacuate gT (+b3) to SBUF right after its g-stream ends,
                while the Vector engine is idle."""
                g_sb = gsbp.tile([128, NDT, OG], fp16, name="g_sb")
                for dt_ in range(NDT):
                    nc.vector.tensor_add(
                        g_sb[:, dt_, :], g_ps_l[dt_][:], b3_t[:]
                    )
                return g_sb

            def out_phase(og, g_sb):
                """scores @ gT for a direct-order o-group."""
                osl = slice(og * OG, (og + 1) * OG)
                for ct in range(NDT):
                    o_ps = po.tile([128, OG], f32, name="o_ps", tag="so")
                    for dt_ in range(NDT):
                        nc.tensor.matmul(
                            o_ps[:],
                            lhsT=sT_sb[:, dt_, ct * 128 : (ct + 1) * 128],
                            rhs=g_sb[:, dt_, :],
                            start=(dt_ == 0),
                            stop=(dt_ == NDT - 1),
                        )
                    out_t = outp.tile([128, OG], f32, name="out_t")
                    nc.vector.tensor_copy(out_t[:], o_ps[:])
                    nc.sync.dma_start(out=out_r[ct, :, osl], in_=out_t[:])

            def bog_begin(og):
                """First half of a reassociated o-group: bias prep + the P8
                fp8-DoubleRow MMs (only need ut8 + the prefetched w3q)."""
                b3_t = b3_tile(og)
                # rank-1 bias term precomputed on DVE; consumed by the
                # PSUM evacuation adds at the end of the stream
                tmp_l = []
                for ct in range(NDT):
                    tmp_t = tmpp.tile([128, OG], bf16, name="tmp_t")
                    nc.vector.tensor_scalar_mul(
                        tmp_t[:], b3_t[:], r_sb[:, ct : ct + 1]
                    )
                    tmp_l.append(tmp_t)
                ps_l = [pg.tile([128, OG], f32, name="g_ps") for _ in range(NDT)]
                w3q_t = w3q_pending.pop(og, None)
                if w3q_t is None:
                    w3q_t = w3q_fetch(og)
                for j in range(P8):
                    for ct in range(NDT):
                        nc.tensor.matmul(
                            ps_l[ct][:],
                            lhsT=ut8_sb[:, j, :, ct * 128 : (ct + 1) * 128],
                            rhs=w3q_t[:, j, :, :],
                            start=(j == 0),
                            stop=False,
                            perf_mode=DR,
                        )
                return ps_l, tmp_l

            def bog_finish(og, ps_l, tmp_l):
                """Second half: the NBF fp16 MMs + evacuation.  The last
                chunk group runs ct-major with per-ct stop + immediate
                evacuation, so three of the four PSUM banks free while the
                tail MMs still stream — the next o-group's first matmuls
                then never wait on this one's evacuation DVE ops."""
                osl = slice(og * OG, (og + 1) * OG)
                mog = og - 2
                plan = _chunk_plan(NBF, NCH)
                m0 = 0
                for gi, ch in enumerate(plan):
                    w3_t = w3p.tile([128, NCH, OG], fp16, name="w3_t")
                    nc.sync.dma_start(
                        out=w3_t[:, :ch, :], in_=w3b_r[:, mog, m0 : m0 + ch, :]
                    )
                    if gi < len(plan) - 1:
                        for j in range(ch):
                            for ct in range(NDT):
                                nc.tensor.matmul(
                                    ps_l[ct][:],
                                    lhsT=ut_sb[:, m0 + j, ct * 128 : (ct + 1) * 128],
                                    rhs=w3_t[:, j, :],
                                    start=False,
                                    stop=False,
                                )
                    else:
                        for ct in range(NDT):
                            for j in range(ch):
                                nc.tensor.matmul(
                                    ps_l[ct][:],
                                    lhsT=ut_sb[:, m0 + j, ct * 128 : (ct + 1) * 128],
                                    rhs=w3_t[:, j, :],
                                    start=False,
                                    stop=(j == ch - 1),
                                )
                            out_t = outp.tile([128, OG], f32, name="out_t")
                            nc.vector.tensor_add(
                                out_t[:], ps_l[ct][:], tmp_l[ct][:]
                            )
                            nc.sync.dma_start(out=out_r[ct, :, osl], in_=out_t[:])
                    m0 += ch

            def bog_phase_ct_major(og):
                """Last o-group runs ct-major: each row tile's accumulation
                completes before the next starts, so its evacuation + out
                DMA overlap the remaining tiles' matmuls (shorter tail)."""
                osl = slice(og * OG, (og + 1) * OG)
                mog = og - 2
                b3_t = b3_tile(og)
                tmp_l = []
                for ct in range(NDT):
                    tmp_t = tmpp.tile([128, OG], bf16, name="tmp_t")
                    nc.vector.tensor_scalar_mul(
                        tmp_t[:], b3_t[:], r_sb[:, ct : ct + 1]
                    )
                    tmp_l.append(tmp_t)
                ps_l = [pg.tile([128, OG], f32, name="g_ps") for _ in range(NDT)]
                w3q_t = w3q_pending.pop(og, None)
                if w3q_t is None:
                    w3q_t = w3q_fetch(og)
                w3_tl = []
                m0 = 0
                for ch in _chunk_plan(NBF, NCH):
                    w3_t = w3p.tile([128, NCH, OG], fp16, name="w3_t")
                    nc.sync.dma_start(
                        out=w3_t[:, :ch, :], in_=w3b_r[:, mog, m0 : m0 + ch, :]
                    )
                    w3_tl.append((w3_t, m0, ch))
                    m0 += ch
                for ct in range(NDT):
                    for j in range(P8):
                        nc.tensor.matmul(
                            ps_l[ct][:],
                            lhsT=ut8_sb[:, j, :, ct * 128 : (ct + 1) * 128],
                            rhs=w3q_t[:, j, :, :],
                            start=(j == 0),
                            stop=False,
                            perf_mode=DR,
                        )
                    for w3_t, m0, ch in w3_tl:
                        for j in range(ch):
                            nc.tensor.matmul(
                                ps_l[ct][:],
                                lhsT=ut_sb[:, m0 + j, ct * 128 : (ct + 1) * 128],
                                rhs=w3_t[:, j, :],
                                start=False,
                                stop=(m0 + j == NBF - 1),
                            )
                    out_t = outp.tile([128, OG], f32, name="out_t")
                    nc.vector.tensor_add(out_t[:], ps_l[ct][:], tmp_l[ct][:])
                    nc.sync.dma_start(out=out_r[ct, :, osl], in_=out_t[:])

            # The PE sits idle ~10us at kernel start waiting for the first
            # DMAs through queue spin-up, and the HAM clock gate holds the
            # PE at 1.2 GHz until it sees ~3.4us of sustained activity.
            # Burn that dead window with tiny N=1 matmuls on a constant so
            # og0's real matmuls start at full clock.
            ones_sb = persist.tile([128, 1], fp16, name="ones_sb")
            nc.vector.memset(ones_sb[:], 1.0)
            warm_ps = pqk.tile([128, C], f32, name="q_ps", tag="qk")
            for i in range(70):
                nc.tensor.matmul(
                    warm_ps[:1, :1],
                    lhsT=ones_sb[:],
                    rhs=ones_sb[:],
                    start=(i == 0),
                    stop=(i == 69),
                )

            # o-group 0's g-stream first, with x1 loads interleaved: the PE
            # starts as soon as the first x1/W3 tile pair lands.
            g0 = g_phase(0, x_load=(x1_sb, x1_r))  # x1 rides og0's stream
            b3_t0 = b3_tile(0)

            # W1T/W2T zero-padded on-chip to 128 output columns: M=128
            # matmuls get fast weight load while only 160KB each moves.
            w1t_sb = persist.tile([128, NI, 128], fp16, name="w1t_sb")
            nc.vector.memset(w1t_sb[:], 0.0)
            nc.sync.dma_start(out=w1t_sb[:, :, :F], in_=w1_r[:])
            w2t_sb = persist.tile([128, NI, 128], fp16, name="w2t_sb")
            nc.vector.memset(w2t_sb[:], 0.0)
            nc.sync.dma_start(out=w2t_sb[:, :, :F], in_=w2_r[:])
            b1_sb = persist.tile([F, 1], f32, name="b1_sb")
            nc.sync.dma_start(out=b1_sb[:], in_=b1[:])
            b2_sb = persist.tile([F, 1], f32, name="b2_sb")
            nc.sync.dma_start(out=b2_sb[:], in_=b2[:])

            # ---- q = (W1/GAMMA) @ (GAMMA x1) + b1 -> [F, C] fp32 ----
            q_ps = pqk.tile([128, C], f32, name="q_ps", tag="qk")
            for n in range(NI):
                nc.tensor.matmul(
                    q_ps[:],
                    lhsT=w1t_sb[:, n, :],
                    rhs=x1_sb[:, n, :],
                    start=(n == 0),
                    stop=(n == NI - 1),
                )
            q_sb = persist.tile([F, C], f32, name="q_sb")
            nc.vector.tensor_scalar_add(q_sb[:], q_ps[:F, :], b1_sb[:])

            # og0's gT evacuates now (Vector is idle; g0 psum is complete)
            g_sb0 = evac_phase(g0, b3_t0)

            # o-group 1's g-stream carries the x2 loads (k runs after it)
            g1 = g_phase(1, x_load=(x2_sb, x2_r))
            b3_t1 = b3_tile(1)

            # ---- k = W2 @ x2 + b2 -> [F, C] fp32 ----
            k_ps = pqk.tile([128, C], f32, name="k_ps", tag="qk")
            for n in range(NI):
                nc.tensor.matmul(
                    k_ps[:],
                    lhsT=w2t_sb[:, n, :],
                    rhs=x2_sb[:, n, :],
                    start=(n == 0),
                    stop=(n == NI - 1),
                )
            k_sb = persist.tile([F, C], f32, name="k_sb")
            nc.vector.tensor_scalar_add(k_sb[:], k_ps[:F, :], b2_sb[:])

            # og1's gT evacuates immediately too
            g_sb1 = evac_phase(g1, b3_t1)

            # x1T streams in while softmax/out-phases run; the u-phase
            # consumes it granule by granule.  og2/og3's fp8 W3 streams
            # prefetch here (interleaved) so og2's DoubleRow matmuls don't
            # stall on DMA right after the u-phase.
            w3q_pending = {}
            for gch in range(NI // NCH):
                nc.sync.dma_start(
                    out=x1t_sb[:, gch * NCH : (gch + 1) * NCH, :, :],
                    in_=x1t_r[:, gch * NCH : (gch + 1) * NCH, :, :],
                )
                if gch == 0:
                    w3q_pending[2] = w3q_fetch(2)
                elif gch == 1:
                    w3q_pending[3] = w3q_fetch(3)

            # ---- sT[d, c] = sum_f k[f,d] q[f,c] (plain fp32 matmul),
            #      then softmax over free (c); emit ALPHA-scaled bf16
            #      scores.  The tiny r-matmuls (r[c] = sum_d sT[d,c],
            #      partition reduce via a ones vector) interleave into the
            #      softmax window. ----
            # four separate PSUM tiles: a column-sliced accumulation in one
            # bank corrupts sibling columns (start=True resets the bank).
            r_ps_l = [pg.tile([128, 1], f32, name="g_ps") for _ in range(NDT)]

            def softmax_tail(dt_, s_ps):
                # logits are bounded (|s| < ~10 for this problem), so plain
                # exp is fp32-safe; skipping the max keeps Exp at one sync
                # wait (the Activation ISA slot allows only one).
                e_sb = small.tile([128, C], f32, name="e_sb")
                esum = small.tile([128, 1], f32, name="esum")
                nc.scalar.activation(
                    e_sb[:], s_ps[:], AF.Exp, scale=1.0, accum_out=esum[:],
                )
                rcp = small.tile([128, 1], f32, name="rcp")
                nc.vector.reciprocal(rcp[:], esum[:])
                rcp_a = small.tile([128, 1], f32, name="rcp_a")
                nc.vector.tensor_scalar_mul(rcp_a[:], rcp[:], float(ALPHA))
                nc.vector.tensor_scalar_mul(sT_sb[:, dt_, :], e_sb[:], rcp_a[:])
                for ct in range(NDT):
                    nc.tensor.matmul(
                        r_ps_l[ct][:],
                        lhsT=sT_sb[:, dt_, ct * 128 : (ct + 1) * 128],
                        rhs=ones_sb[:],
                        start=(dt_ == 0),
                        stop=(dt_ == NDT - 1),
                    )

            s_pend = None
            for dt_ in range(NDT):
                s_ps = po.tile([128, C], f32, name="s_ps", tag="so")
                nc.tensor.matmul(
                    s_ps[:],
                    lhsT=k_sb[:, dt_ * 128 : (dt_ + 1) * 128],
                    rhs=q_sb[:],
                    start=True,
                    stop=True,
                )
                if s_pend is not None:
                    softmax_tail(dt_ - 1, s_pend)
                s_pend = s_ps
            softmax_tail(NDT - 1, s_pend)
            r_sb = persist.tile([128, NDT], f32, name="r_sb")
            for ct in range(NDT):
                nc.vector.tensor_copy(r_sb[:, ct : ct + 1], r_ps_l[ct][:])

            # ---- direct-order output for o-groups 0/1 ----
            out_phase(0, g_sb0)
            out_phase(1, g_sb1)

            # ---- u-phase: uT[i, c] = ALPHA * sum_d x1[i,d] s[c,d] ----
            # first NF8 chunks quantize to fp8e4 (clipped +-224 via the two
            # tensor_scalar alu slots); the rest evacuate fp16.
            def u_chunk(n):
                u_ps = po.tile([128, C], f32, name="u_ps", tag="so")
                for dt_ in range(NDT):
                    nc.tensor.matmul(
                        u_ps[:],
                        lhsT=x1t_sb[:, n, dt_, :],
                        rhs=sT_sb[:, dt_, :],
                        start=(dt_ == 0),
                        stop=(dt_ == NDT - 1),
                    )
                if n < NF8:
                    nc.vector.tensor_scalar(
                        ut8_sb[:, n // 2, n % 2, :], u_ps[:],
                        224.0, -224.0, ALU.min, ALU.max,
                    )
                else:
                    nc.vector.tensor_copy(ut_sb[:, n - NF8, :], u_ps[:])

            for n in range(NF8):
                u_chunk(n)
            # og2's fp8-DoubleRow block interleaves here: its inputs (ut8 +
            # the prefetched w3q) are ready, and it gives the x1t stream
            # breathing room so the tail u-chunks don't stall on DMA.
            og2_ps, og2_tmp = bog_begin(2)
            # The tail u-chunks and og2's fp16 stream merge, staggered by
            # one chunk (og2's block m needs ut[m], evacuated one round
            # earlier): halves the x1t consumption rate so its DMA stream
            # keeps pace, and keeps the PE dense through this window.
            u_chunk(NF8)
            osl2 = slice(2 * OG, 3 * OG)
            plan2 = _chunk_plan(NBF, NCH)
            gi = 0
            j_in = 0
            ch2 = plan2[0]
            w3_t2 = None
            for m in range(NBF):
                if j_in == 0:
                    ch2 = plan2[gi]
                    w3_t2 = w3p.tile([128, NCH, OG], fp16, name="w3_t")
                    nc.sync.dma_start(
                        out=w3_t2[:, :ch2, :], in_=w3b_r[:, 0, m : m + ch2, :]
                    )
                if NF8 + m + 1 < NI:
                    u_chunk(NF8 + m + 1)
                for ct in range(NDT):
                    nc.tensor.matmul(
                        og2_ps[ct][:],
                        lhsT=ut_sb[:, m, ct * 128 : (ct + 1) * 128],
                        rhs=w3_t2[:, j_in, :],
                        start=False,
                        stop=(m == NBF - 1),
                    )
                j_in += 1
                if j_in == ch2:
                    j_in = 0
                    gi += 1
            for ct in range(NDT):
                out_t = outp.tile([128, OG], f32, name="out_t")
                nc.vector.tensor_add(out_t[:], og2_ps[ct][:], og2_tmp[ct][:])
                nc.sync.dma_start(out=out_r[ct, :, osl2], in_=out_t[:])

            # ---- main: reassociated hybrid stream for o-groups 3..15 ----
            for og in range(3, N_OG - 1):
                ps_l, tmp_l = bog_begin(og)
                bog_finish(og, ps_l, tmp_l)
            bog_phase_ct_major(N_OG - 1)

    nc.finalize()
    return nc


def _get_nc():
    if "nc" not in _NC_CACHE:
        _NC_CACHE["nc"] = _build_nc()
    return _NC_CACHE["nc"]


def _stage_inputs(input1, input2, W1, b1, W2, b2, W3, b3):
    input1 = np.asarray(input1, np.float32)
    input2 = np.asarray(input2, np.float32)
    W1 = np.asarray(W1, np.float32)
    W2 = np.asarray(W2, np.float32)
    W3 = np.asarray(W3, np.float32)
    b1 = np.asarray(b1, np.float32)
    b2 = np.asarray(b2, np.float32)
    b3 = np.asarray(b3, np.float32)

    def pmajor(X, inner):
        # [THW, inner] -> [128, NI*inner]: row p = concat_n X[n*128+p, :]
        return np.ascontiguousarray(
            X.reshape(-1, 128, inner).transpose(1, 0, 2).reshape(128, -1)
        )

    # [B,T,C,H,W] -> x[b][i=(t,hw), c], partition-major
    X1f = np.ascontiguousarray(
        input1.reshape(B, T, C, H * W).transpose(0, 1, 3, 2)
    ).reshape(B, THW, C)
    X2f = np.ascontiguousarray(
        input2.reshape(B, T, C, H * W).transpose(0, 1, 3, 2)
    ).reshape(B, THW, C)
    X1g = (GAMMA * X1f).astype(FP16NP)          # gamma-scaled fp16
    X2 = X2f.astype(FP16NP)
    X1p = [pmajor(X1g[b], C) for b in range(B)]
    X2p = [pmajor(X2[b], C) for b in range(B)]
    # channel-major x1 (unscaled): [128p, (n, dt, j)] = x1[n*128+j, dt*128+p]
    X1b = X1f.astype(FP16NP)
    X1Tp = [
        np.ascontiguousarray(
            X1b[b].reshape(NI, 128, NDT, 128).transpose(3, 0, 2, 1)
        ).reshape(128, NI * NDT * 128)
        for b in range(B)
    ]
    W1Tp = pmajor(np.ascontiguousarray((W1 / GAMMA).T).astype(FP16NP), F)
    W2Tp = pmajor(np.ascontiguousarray(W2.T).astype(FP16NP), F)
    W3T = np.ascontiguousarray(W3.T)             # [THW, O_TOT]
    W3q8 = (BETA * W3T[: NF8 * 128]).astype(E4M3NP)   # fp8 rows (main ogs)
    W3bb = (BETA * W3T).astype(FP16NP)           # fp16 rows, main ogs
    W3gg = (BETA01 * W3T).astype(FP16NP)         # fp16 rows, og0/1
    b1c = np.ascontiguousarray(b1.reshape(F, 1))
    b2c = np.ascontiguousarray(b2.reshape(F, 1))

    in_maps = []
    for core in range(8):
        b = core // 2
        half = core % 2
        osl = slice(half * O_HALF, (half + 1) * O_HALF)
        W3h8 = W3q8[:, osl]
        W3hb = W3bb[:, osl]
        W3hg = W3gg[:, osl]
        # og0/1: full-depth fp16 [128, (og2, n40, oc)]
        w3g_core = np.ascontiguousarray(
            W3hg[:, : 2 * OG]
            .reshape(NI, 128, 2, OG)
            .transpose(1, 2, 0, 3)
            .reshape(128, 2 * NI * OG)
        )
        # main ogs fp8 pairs -> [128, (og, j, s, oc)]
        w3q_core = np.ascontiguousarray(
            W3h8[:, 2 * OG :]
            .reshape(P8, 2, 128, N_MOG, OG)
            .transpose(2, 3, 0, 1, 4)
            .reshape(128, N_MOG * P8 * 2 * OG)
        )
        # main ogs bf16 tail -> [128, (og, m, oc)]
        w3b_core = np.ascontiguousarray(
            W3hb[NF8 * 128 :, 2 * OG :]
            .reshape(NBF, 128, N_MOG, OG)
            .transpose(1, 2, 0, 3)
            .reshape(128, N_MOG * NBF * OG)
        )
        b3h = b3[osl]
        in_maps.append(
            {
                "x1": X1p[b],
                "x2": X2p[b],
                "x1t": X1Tp[b],
                "w1t": W1Tp,
                "w2t": W2Tp,
                "w3g": w3g_core,
                "w3q": w3q_core,
                "w3b": w3b_core,
                "b1": b1c,
                "b2": b2c,
                "b3b": np.ascontiguousarray(
                    np.broadcast_to(
                        (BETA * b3h).astype(BF16NP)[None, :], (128, O_HALF)
                    )
                ),
                "b3g": np.ascontiguousarray(
                    np.broadcast_to(
                        (GAMMA * BETA01 * b3h[: 2 * OG]).astype(FP16NP)[None, :],
                        (128, 2 * OG),
                    )
                ),
            }
        )
    return in_maps


def run(inputs: dict, trace: bool = False):
    """Returns (full_output [B,F,C,H,W], BassKernelResults)."""
    in_maps = _stage_inputs(**inputs)
    nc = _get_nc()
    res = run_bass_kernel_spmd(nc, in_maps, core_ids=list(range(8)), trace=trace)
    out_full = np.empty((B, C, O_TOT), np.float32)
    for core in range(8):
        b = core // 2
        half = core % 2
        out_full[b, :, half * O_HALF : (half + 1) * O_HALF] = res.results[core]["out"]
    # host unscale: og0/1 of each half at ALPHA*GAMMA*BETA, rest ALPHA*BETA
    inv_main = 1.0 / (ALPHA * BETA)
    inv_01 = 1.0 / (ALPHA * GAMMA * BETA01)
    for half in range(2):
        lo = half * O_HALF
        out_full[:, :, lo : lo + 2 * OG] *= inv_01
        out_full[:, :, lo + 2 * OG : lo + O_HALF] *= inv_main
    out = np.ascontiguousarray(
        out_full.reshape(B, C, F, H, W).transpose(0, 2, 1, 3, 4)
    )
    return out, res


def kernel(**inputs) -> np.ndarray:
    out, _ = run(inputs, trace=False)
    return out


# revision 18
# speedup vs baseline: 1.0059x; 1.0021x over previous
"""Trainium2 Bass kernel for nn_Channel_dot — hybrid fp8-DoubleRow/bf16.

Math (per batch b):
  x1 = reshape(input1) -> [THW, C];  x2 likewise
  q  = W1 @ x1 + b1            [F, C]
  k  = W2 @ x2 + b2            [F, C]
  sT = k^T q                   [C(d), C(c)]  (sT[d,c] = s[c,d])
  scoresT = softmax over c (free axis of sT)   -- fp32
  out[c,o] = sum_d s[c,d] * (W3 @ x1 + b3)[o,d]
           = sum_i uT[i,c] * W3T[i,o] + r[c]*b3[o]
    where uT[i,c] = sum_d x1[i,d] sT[d,c],  r[c] = sum_d sT[d,c]

Sharding: 8 cores = 4 batches x 2 halves of the G3 output dim (O=16384).

o-groups 0/1 run the direct order (gT = x1^T W3T streamed, then s @ gT)
in pure bf16 — they keep the PE busy while x1/x2 stream in and
q/k/softmax resolve.  o-groups 2..15 use the reassociated order, with
the first 2*P8 i-chunks (of 40) as P8 fp8e4 DoubleRow matmuls (2 chunks
per MM at the same 216ns as one bf16 MM — measured on HW) and the rest
bf16.  P8 is calibrated so global rel-err stays under the 2e-2 gate.

Scale algebra (powers of two, exact): sT is produced ALPHA-scaled, so u
arrives in PSUM as ALPHA*u and casts straight to fp8e4 (clipped +-224
via the two tensor_scalar alu slots); W3 is staged BETA-scaled (fp8
rows e4m3, bf16 rows an exact exponent shift) so fp8 and bf16 chunks
accumulate in ONE psum bank at scale ALPHA*BETA.  og0/1 use GAMMA-
scaled x1; the host divides the output by the per-region scale at the
end.
"""

import os
import sys

for _p in ("/opt/trn_rl_repo", "/root/.axon_site/_ro/trn_rl_repo"):
    if os.path.isdir(_p) and _p not in sys.path:
        sys.path.insert(0, _p)

import numpy as np
import ml_dtypes

import concourse.bacc as bacc
import concourse.bass as bass
import concourse.mybir as mybir
import concourse.tile as tile
from concourse.bass_utils import run_bass_kernel_spmd

B, T, C, H, W = 4, 5, 512, 32, 32
F = 16
THW = T * H * W            # 5120
O_TOT = F * H * W          # 16384
O_HALF = O_TOT // 2        # 8192 per core
NI = THW // 128            # 40 i-chunks
OG = 512                   # o-columns per inner group (1 PSUM bank)
N_OG = O_HALF // OG        # 16
N_MOG = N_OG - 2           # reassociated (main) o-groups
NDT = C // 128             # 4 channel tiles

P8 = 8                     # fp8 pair-chunks per (main og, ct)
NF8 = 2 * P8               # i-chunks covered by fp8
NBF = NI - NF8             # bf16 i-chunks in main o-groups
ALPHA = 2.0 ** 5           # sT scale (alpha*u max ~210 < 240)
BETA = 2.0 ** 13           # W3 scale (beta*W3 max ~115 < 240)
BETA01 = 2.0 ** 8          # W3 scale for og0/1 (g_sb must fit fp16)
GAMMA = 2.0 ** 5           # x1 scale for og0/1 direct path

f32 = mybir.dt.float32
bf16 = mybir.dt.bfloat16
fp16 = mybir.dt.float16
fp8e4 = mybir.dt.float8e4
DR = mybir.MatmulPerfMode.DoubleRow
AF = mybir.ActivationFunctionType
AX = mybir.AxisListType
ALU = mybir.AluOpType
BF16NP = np.dtype(ml_dtypes.bfloat16)
FP16NP = np.dtype(np.float16)
E4M3NP = np.dtype(ml_dtypes.float8_e4m3)

_NC_CACHE = {}


def _chunk_plan(total, step):
    # near-equal parts, each <= step (DMA descriptor batching)
    k = -(-total // step)
    base, rem = divmod(total, k)
    return [base + (i < rem) for i in range(k)]


def _build_nc():
    # Bacc (not plain Bass): its finalize() runs generate_event_semaphores(),
    # which splits multi-wait sync onto EventSemaphore ops — TRN2 compute
    # instructions encode at most one sync wait.
    nc = bacc.Bacc()

    # All streamed inputs are staged partition-major on the host so each
    # DMA reads multi-KB contiguous runs per partition.
    NCH = 5                     # i-chunks per DMA chunk
    x1 = nc.dram_tensor("x1", [128, NI * C], fp16, kind="ExternalInput")
    x2 = nc.dram_tensor("x2", [128, NI * C], fp16, kind="ExternalInput")
    # x1 transposed to channel-major: x1t[p, (n, dt, j)] = x1[i=n*128+j,
    # c=dt*128+p] — the u-phase contracts over the channel dim.
    x1t = nc.dram_tensor("x1t", [128, NI * NDT * 128], fp16, kind="ExternalInput")
    w1t = nc.dram_tensor("w1t", [128, NI * F], fp16, kind="ExternalInput")
    w2t = nc.dram_tensor("w2t", [128, NI * F], fp16, kind="ExternalInput")
    # W3 streams: og0/1 full-bf16; main ogs split fp8-pairs + bf16 tail.
    w3g = nc.dram_tensor("w3g", [128, 2 * NI * OG], fp16, kind="ExternalInput")
    w3q = nc.dram_tensor("w3q", [128, N_MOG * P8 * 2 * OG], fp8e4,
                         kind="ExternalInput")
    w3b = nc.dram_tensor("w3b", [128, N_MOG * NBF * OG], fp16,
                         kind="ExternalInput")
    b1 = nc.dram_tensor("b1", [F, 1], f32, kind="ExternalInput")
    b2 = nc.dram_tensor("b2", [F, 1], f32, kind="ExternalInput")
    # b3 replicated to 128 partitions on the host; streamed per o-group.
    b3b = nc.dram_tensor("b3b", [128, O_HALF], bf16, kind="ExternalInput")
    b3g = nc.dram_tensor("b3g", [128, 2 * OG], fp16, kind="ExternalInput")
    out = nc.dram_tensor("out", [C, O_HALF], f32, kind="ExternalOutput")

    x1_r = x1.rearrange("p (n c) -> p n c", c=C)
    x2_r = x2.rearrange("p (n c) -> p n c", c=C)
    x1t_r = x1t.rearrange("p (n dt j) -> p n dt j", dt=NDT, j=128)
    w3g_r = w3g.rearrange("p (og n oc) -> p og n oc", og=2, n=NI)
    w3q_r = w3q.rearrange("p (og j s oc) -> p og j s oc", og=N_MOG, j=P8, s=2)
    w3b_r = w3b.rearrange("p (og m oc) -> p og m oc", og=N_MOG, m=NBF)
    w1_r = w1t.rearrange("p (n f) -> p n f", f=F)
    w2_r = w2t.rearrange("p (n f) -> p n f", f=F)
    out_r = out.rearrange("(ct p) o -> ct p o", p=128)

    with tile.TileContext(nc) as tc:
        with (
            tc.tile_pool(name="persist", bufs=1) as persist,
            tc.tile_pool(name="w3p", bufs=7) as w3p,
            tc.tile_pool(name="w3qp", bufs=3) as w3qp,
            tc.tile_pool(name="gsbp", bufs=2) as gsbp,
            tc.tile_pool(name="outp", bufs=4) as outp,
            tc.tile_pool(name="b3p", bufs=3) as b3p,
            tc.tile_pool(name="tmpp", bufs=8) as tmpp,
            tc.tile_pool(name="small", bufs=2) as small,
            tc.tile_pool(name="pg", bufs=5, space="PSUM") as pg,
            tc.tile_pool(name="po", bufs=2, space="PSUM") as po,
            tc.tile_pool(name="pqk", bufs=1, space="PSUM") as pqk,
        ):
            # ---- persistent tiles ----
            # x1 (i-major) is dead after og0/1's g-streams + q; the
            # x1T layout reuses its SBUF slot via the shared tag.
            x1_sb = persist.tile([128, NI, C], fp16, name="x1_sb", tag="x1x")
            x1t_sb = persist.tile(
                [128, NI, NDT, 128], fp16, name="x1t_sb", tag="x1x"
            )
            # x2 is dead after k; uT (bf16 + fp8 parts) reuses its slot.
            x2_sb = persist.tile([128, NI, C], fp16, name="x2_sb", tag="xu")
            ut_sb = persist.tile([128, NBF, C], fp16, name="ut_sb", tag="xu")
            ut8_sb = persist.tile([128, P8, 2, C], fp8e4, name="ut8_sb")
            sT_sb = persist.tile([128, NDT, C], fp16, name="sT_sb")

            def b3_tile(og):
                dt_b3 = fp16 if og < 2 else bf16
                b3_t = b3p.tile([128, OG], dt_b3, name="b3_t")
                src_t = b3g if og < 2 else b3b
                nc.sync.dma_start(out=b3_t[:], in_=src_t[:, og * OG : (og + 1) * OG])
                return b3_t

            def w3q_fetch(og):
                w3q_t = w3qp.tile([128, P8, 2, OG], fp8e4, name="w3q_t")
                nc.sync.dma_start(out=w3q_t[:], in_=w3q_r[:, og - 2])
                return w3q_t

            def g_phase(og, x_load=None):
                """Direct-order o-group (0/1): stream W3 columns, accumulate
                gT = (GAMMA x1)^T (BETA W3T) in PSUM, pure bf16."""
                g_ps_l = [pg.tile([128, OG], f32, name="g_ps") for _ in range(NDT)]
                # og 0 ramps with fine-grained chunks so the very first
                # matmul starts as early as possible (DMA queues are still
                # spinning up during the first ~15us)
                plan = [1, 1, 2, 3, 4, 4, 5, 5, 5, 5, 5] if og == 0 else \
                    _chunk_plan(NI, NCH)
                n0 = 0
                for ch in plan:
                    if x_load is not None:
                        # one x chunk rides along per w3 chunk so the
                        # prologue inputs arrive without their own phase
                        nc.sync.dma_start(
                            out=x_load[0][:, n0 : n0 + ch, :],
                            in_=x_load[1][:, n0 : n0 + ch, :],
                        )
                    w3_t = w3p.tile([128, NCH, OG], fp16, name="w3_t")
                    nc.sync.dma_start(
                        out=w3_t[:, :ch, :], in_=w3g_r[:, og, n0 : n0 + ch, :]
                    )
                    for j in range(ch):
                        for dt_ in range(NDT):
                            nc.tensor.matmul(
                                g_ps_l[dt_][:],
                                lhsT=x1_sb[:, n0 + j, dt_ * 128 : (dt_ + 1) * 128],
                                rhs=w3_t[:, j, :],
                                start=(n0 + j == 0),
                                stop=(n0 + j == NI - 1),
                            )
                    n0 += ch
                return g_ps_l

            def evac_phase(g_ps_l, b3_t):
                """Evacuate gT (+b3) to SBUF right after its g-stream ends,
                while the Vector engine is idle."""
                g_sb = gsbp.tile([128, NDT, OG], fp16, name="g_sb")
                for dt_ in range(NDT):
                    nc.vector.tensor_add(
                        g_sb[:, dt_, :], g_ps_l[dt_][:], b3_t[:]
                    )
                return g_sb

            def out_phase(og, g_sb):
                """scores @ gT for a direct-order o-group."""
                osl = slice(og * OG, (og + 1) * OG)
                for ct in range(NDT):
                    o_ps = po.tile([128, OG], f32, name="o_ps", tag="so")
                    for dt_ in range(NDT):
                        nc.tensor.matmul(
                            o_ps[:],
                            lhsT=sT_sb[:, dt_, ct * 128 : (ct + 1) * 128],
                            rhs=g_sb[:, dt_, :],
                            start=(dt_ == 0),
                            stop=(dt_ == NDT - 1),
                        )
                    out_t = outp.tile([128, OG], f32, name="out_t")
                    nc.vector.tensor_copy(out_t[:], o_ps[:])
                    nc.sync.dma_start(out=out_r[ct, :, osl], in_=out_t[:])

            def bog_begin(og):
                """First half of a reassociated o-group: bias prep + the P8
                fp8-DoubleRow MMs (only need ut8 + the prefetched w3q)."""
                b3_t = b3_tile(og)
                # rank-1 bias term precomputed on DVE; consumed by the
                # PSUM evacuation adds at the end of the stream
                tmp_l = []
                for ct in range(NDT):
                    tmp_t = tmpp.tile([128, OG], bf16, name="tmp_t")
                    nc.vector.tensor_scalar_mul(
                        tmp_t[:], b3_t[:], r_sb[:, ct : ct + 1]
                    )
                    tmp_l.append(tmp_t)
                ps_l = [pg.tile([128, OG], f32, name="g_ps") for _ in range(NDT)]
                w3q_t = w3q_pending.pop(og, None)
                if w3q_t is None:
                    w3q_t = w3q_fetch(og)
                for j in range(P8):
                    for ct in range(NDT):
                        nc.tensor.matmul(
                            ps_l[ct][:],
                            lhsT=ut8_sb[:, j, :, ct * 128 : (ct + 1) * 128],
                            rhs=w3q_t[:, j, :, :],
                            start=(j == 0),
                            stop=False,
                            perf_mode=DR,
                        )
                return ps_l, tmp_l

            def bog_finish(og, ps_l, tmp_l):
                """Second half: the NBF fp16 MMs + evacuation.  The last
                chunk group runs ct-major with per-ct stop + immediate
                evacuation, so three of the four PSUM banks free while the
                tail MMs still stream — the next o-group's first matmuls
                then never wait on this one's evacuation DVE ops."""
                osl = slice(og * OG, (og + 1) * OG)
                mog = og - 2
                plan = _chunk_plan(NBF, NCH)
                m0 = 0
                for gi, ch in enumerate(plan):
                    w3_t = w3p.tile([128, NCH, OG], fp16, name="w3_t")
                    nc.sync.dma_start(
                        out=w3_t[:, :ch, :], in_=w3b_r[:, mog, m0 : m0 + ch, :]
                    )
                    if gi < len(plan) - 1:
                        for j in range(ch):
                            for ct in range(NDT):
                                nc.tensor.matmul(
                                    ps_l[ct][:],
                                    lhsT=ut_sb[:, m0 + j, ct * 128 : (ct + 1) * 128],
                                    rhs=w3_t[:, j, :],
                                    start=False,
                                    stop=False,
                                )
                    else:
                        for ct in range(NDT):
                            for j in range(ch):
                                nc.tensor.matmul(
                                    ps_l[ct][:],
                                    lhsT=ut_sb[:, m0 + j, ct * 128 : (ct + 1) * 128],
                                    rhs=w3_t[:, j, :],
                                    start=False,
                                    stop=(j == ch - 1),
                                )
                            out_t = outp.tile([128, OG], f32, name="out_t")
                            nc.vector.tensor_add(
                                out_t[:], ps_l[ct][:], tmp_l[ct][:]
                            )
                            nc.sync.dma_start(out=out_r[ct, :, osl], in_=out_t[:])
                    m0 += ch

            def bog_phase_ct_major(og):
                """Last o-group runs ct-major: each row tile's accumulation
                completes before the next starts, so its evacuation + out
                DMA overlap the remaining tiles' matmuls (shorter tail)."""
                osl = slice(og * OG, (og + 1) * OG)
                mog = og - 2
                b3_t = b3_tile(og)
                tmp_l = []
                for ct in range(NDT):
                    tmp_t = tmpp.tile([128, OG], bf16, name="tmp_t")
                    nc.vector.tensor_scalar_mul(
                        tmp_t[:], b3_t[:], r_sb[:, ct : ct + 1]
                    )
                    tmp_l.append(tmp_t)
                ps_l = [pg.tile([128, OG], f32, name="g_ps") for _ in range(NDT)]
                w3q_t = w3q_pending.pop(og, None)
                if w3q_t is None:
                    w3q_t = w3q_fetch(og)
                w3_tl = []
                m0 = 0
                for ch in _chunk_plan(NBF, NCH):
                    w3_t = w3p.tile([128, NCH, OG], fp16, name="w3_t")
                    nc.sync.dma_start(
                        out=w3_t[:, :ch, :], in_=w3b_r[:, mog, m0 : m0 + ch, :]
                    )
                    w3_tl.append((w3_t, m0, ch))
                    m0 += ch
                for ct in range(NDT):
                    for j in range(P8):
                        nc.tensor.matmul(
                            ps_l[ct][:],
                            lhsT=ut8_sb[:, j, :, ct * 128 : (ct + 1) * 128],
                            rhs=w3q_t[:, j, :, :],
                            start=(j == 0),
                            stop=False,
                            perf_mode=DR,
                        )
                    for w3_t, m0, ch in w3_tl:
                        for j in range(ch):
                            nc.tensor.matmul(
                                ps_l[ct][:],
                                lhsT=ut_sb[:, m0 + j, ct * 128 : (ct + 1) * 128],
                                rhs=w3_t[:, j, :],
                                start=False,
                                stop=(m0 + j == NBF - 1),
                            )
                    out_t = outp.tile([128, OG], f32, name="out_t")
                    nc.vector.tensor_add(out_t[:], ps_l[ct][:], tmp_l[ct][:])
                    nc.sync.dma_start(out=out_r[ct, :, osl], in_=out_t[:])

            # The PE sits idle ~10us at kernel start waiting for the first
            # DMAs through queue spin-up, and the HAM clock gate holds the
            # PE at 1.2 GHz until it sees ~3.4us of sustained activity.
            # Burn that dead window with tiny N=1 matmuls on a constant so
            # og0's real matmuls start at full clock.
            ones_sb = persist.tile([128, 1], fp16, name="ones_sb")
            nc.vector.memset(ones_sb[:], 1.0)
            warm_sb = persist.tile([128, 128], fp16, name="warm_sb")
            nc.vector.memset(warm_sb[:], 1.0)
            warm_ps = pqk.tile([128, C], f32, name="q_ps", tag="qk")
            for i in range(36):
                nc.tensor.matmul(
                    warm_ps[:, :128],
                    lhsT=warm_sb[:],
                    rhs=warm_sb[:],
                    start=(i == 0),
                    stop=(i == 35),
                )

            # o-group 0's g-stream first, with x1 loads interleaved: the PE
            # starts as soon as the first x1/W3 tile pair lands.
            g0 = g_phase(0, x_load=(x1_sb, x1_r))  # x1 rides og0's stream
            b3_t0 = b3_tile(0)

            # W1T/W2T zero-padded on-chip to 128 output columns: M=128
            # matmuls get fast weight load while only 160KB each moves.
            w1t_sb = persist.tile([128, NI, 128], fp16, name="w1t_sb")
            nc.vector.memset(w1t_sb[:], 0.0)
            nc.sync.dma_start(out=w1t_sb[:, :, :F], in_=w1_r[:])
            w2t_sb = persist.tile([128, NI, 128], fp16, name="w2t_sb")
            nc.vector.memset(w2t_sb[:], 0.0)
            nc.sync.dma_start(out=w2t_sb[:, :, :F], in_=w2_r[:])
            b1_sb = persist.tile([F, 1], f32, name="b1_sb")
            nc.sync.dma_start(out=b1_sb[:], in_=b1[:])
            b2_sb = persist.tile([F, 1], f32, name="b2_sb")
            nc.sync.dma_start(out=b2_sb[:], in_=b2[:])

            # ---- q = (W1/GAMMA) @ (GAMMA x1) + b1 -> [F, C] fp32 ----
            q_ps = pqk.tile([128, C], f32, name="q_ps", tag="qk")
            for n in range(NI):
                nc.tensor.matmul(
                    q_ps[:],
                    lhsT=w1t_sb[:, n, :],
                    rhs=x1_sb[:, n, :],
                    start=(n == 0),
                    stop=(n == NI - 1),
                )
            q_sb = persist.tile([F, C], f32, name="q_sb")
            nc.vector.tensor_scalar_add(q_sb[:], q_ps[:F, :], b1_sb[:])

            # og0's gT evacuates now (Vector is idle; g0 psum is complete)
            g_sb0 = evac_phase(g0, b3_t0)

            # o-group 1's g-stream carries the x2 loads (k runs after it)
            g1 = g_phase(1, x_load=(x2_sb, x2_r))
            b3_t1 = b3_tile(1)

            # ---- k = W2 @ x2 + b2 -> [F, C] fp32 ----
            k_ps = pqk.tile([128, C], f32, name="k_ps", tag="qk")
            for n in range(NI):
                nc.tensor.matmul(
                    k_ps[:],
                    lhsT=w2t_sb[:, n, :],
                    rhs=x2_sb[:, n, :],
                    start=(n == 0),
                    stop=(n == NI - 1),
                )
            k_sb = persist.tile([F, C], f32, name="k_sb")
            nc.vector.tensor_scalar_add(k_sb[:], k_ps[:F, :], b2_sb[:])

            # og1's gT evacuates immediately too
            g_sb1 = evac_phase(g1, b3_t1)

            # x1T streams in while softmax/out-phases run; the u-phase
            # consumes it granule by granule.  og2/og3's fp8 W3 streams
            # prefetch here (interleaved) so og2's DoubleRow matmuls don't
            # stall on DMA right after the u-phase.
            w3q_pending = {}
            for gch in range(NI // NCH):
                nc.sync.dma_start(
                    out=x1t_sb[:, gch * NCH : (gch + 1) * NCH, :, :],
                    in_=x1t_r[:, gch * NCH : (gch + 1) * NCH, :, :],
                )
                if gch == 0:
                    w3q_pending[2] = w3q_fetch(2)
                elif gch == 1:
                    w3q_pending[3] = w3q_fetch(3)

            # ---- sT[d, c] = sum_f k[f,d] q[f,c] (plain fp32 matmul),
            #      then softmax over free (c); emit ALPHA-scaled bf16
            #      scores.  The tiny r-matmuls (r[c] = sum_d sT[d,c],
            #      partition reduce via a ones vector) interleave into the
            #      softmax window. ----
            # four separate PSUM tiles: a column-sliced accumulation in one
            # bank corrupts sibling columns (start=True resets the bank).
            r_ps_l = [pg.tile([128, 1], f32, name="g_ps") for _ in range(NDT)]

            def softmax_tail(dt_, s_ps):
                # logits are bounded (|s| < ~10 for this problem), so plain
                # exp is fp32-safe; skipping the max keeps Exp at one sync
                # wait (the Activation ISA slot allows only one).
                e_sb = small.tile([128, C], f32, name="e_sb")
                esum = small.tile([128, 1], f32, name="esum")
                nc.scalar.activation(
                    e_sb[:], s_ps[:], AF.Exp, scale=1.0, accum_out=esum[:],
                )
                rcp = small.tile([128, 1], f32, name="rcp")
                nc.vector.reciprocal(rcp[:], esum[:])
                rcp_a = small.tile([128, 1], f32, name="rcp_a")
                nc.vector.tensor_scalar_mul(rcp_a[:], rcp[:], float(ALPHA))
                nc.vector.tensor_scalar_mul(sT_sb[:, dt_, :], e_sb[:], rcp_a[:])
                for ct in range(NDT):
                    nc.tensor.matmul(
                        r_ps_l[ct][:],
                        lhsT=sT_sb[:, dt_, ct * 128 : (ct + 1) * 128],
                        rhs=ones_sb[:],
                        start=(dt_ == 0),
                        stop=(dt_ == NDT - 1),
                    )

            s_pend = None
            for dt_ in range(NDT):
                s_ps = po.tile([128, C], f32, name="s_ps", tag="so")
                nc.tensor.matmul(
                    s_ps[:],
                    lhsT=k_sb[:, dt_ * 128 : (dt_ + 1) * 128],
                    rhs=q_sb[:],
                    start=True,
                    stop=True,
                )
                if s_pend is not None:
                    softmax_tail(dt_ - 1, s_pend)
                s_pend = s_ps
            softmax_tail(NDT - 1, s_pend)
            r_sb = persist.tile([128, NDT], f32, name="r_sb")
            for ct in range(NDT):
                nc.vector.tensor_copy(r_sb[:, ct : ct + 1], r_ps_l[ct][:])

            # ---- direct-order output for o-groups 0/1 ----
            out_phase(0, g_sb0)
            out_phase(1, g_sb1)

            # ---- u-phase: uT[i, c] = ALPHA * sum_d x1[i,d] s[c,d] ----
            # first NF8 chunks quantize to fp8e4 (clipped +-224 via the two
            # tensor_scalar alu slots); the rest evacuate fp16.
            def u_chunk(n):
                u_ps = po.tile([128, C], f32, name="u_ps", tag="so")
                for dt_ in range(NDT):
                    nc.tensor.matmul(
                        u_ps[:],
                        lhsT=x1t_sb[:, n, dt_, :],
                        rhs=sT_sb[:, dt_, :],
                        start=(dt_ == 0),
                        stop=(dt_ == NDT - 1),
                    )
                if n < NF8:
                    nc.vector.tensor_scalar(
                        ut8_sb[:, n // 2, n % 2, :], u_ps[:],
                        224.0, -224.0, ALU.min, ALU.max,
                    )
                else:
                    nc.vector.tensor_copy(ut_sb[:, n - NF8, :], u_ps[:])

            for n in range(NF8):
                u_chunk(n)
            # og2's fp8-DoubleRow block interleaves here: its inputs (ut8 +
            # the prefetched w3q) are ready, and it gives the x1t stream
            # breathing room so the tail u-chunks don't stall on DMA.
            og2_ps, og2_tmp = bog_begin(2)
            # The tail u-chunks and og2's fp16 stream merge, staggered by
            # one chunk (og2's block m needs ut[m], evacuated one round
            # earlier): halves the x1t consumption rate so its DMA stream
            # keeps pace, and keeps the PE dense through this window.
            u_chunk(NF8)
            osl2 = slice(2 * OG, 3 * OG)
            plan2 = _chunk_plan(NBF, NCH)
            gi = 0
            j_in = 0
            ch2 = plan2[0]
            w3_t2 = None
            for m in range(NBF):
                if j_in == 0:
                    ch2 = plan2[gi]
                    w3_t2 = w3p.tile([128, NCH, OG], fp16, name="w3_t")
                    nc.sync.dma_start(
                        out=w3_t2[:, :ch2, :], in_=w3b_r[:, 0, m : m + ch2, :]
                    )
                if NF8 + m + 1 < NI:
                    u_chunk(NF8 + m + 1)
                for ct in range(NDT):
                    nc.tensor.matmul(
                        og2_ps[ct][:],
                        lhsT=ut_sb[:, m, ct * 128 : (ct + 1) * 128],
                        rhs=w3_t2[:, j_in, :],
                        start=False,
                        stop=(m == NBF - 1),
                    )
                j_in += 1
                if j_in == ch2:
                    j_in = 0
                    gi += 1
            for ct in range(NDT):
                out_t = outp.tile([128, OG], f32, name="out_t")
                nc.vector.tensor_add(out_t[:], og2_ps[ct][:], og2_tmp[ct][:])
                nc.sync.dma_start(out=out_r[ct, :, osl2], in_=out_t[:])

            # ---- main: reassociated hybrid stream for o-groups 3..15 ----
            for og in range(3, N_OG - 1):
                ps_l, tmp_l = bog_begin(og)
                bog_finish(og, ps_l, tmp_l)
            bog_phase_ct_major(N_OG - 1)

    nc.finalize()
    return nc


def _get_nc():
    if "nc" not in _NC_CACHE:
        _NC_CACHE["nc"] = _build_nc()
    return _NC_CACHE["nc"]


def _stage_inputs(input1, input2, W1, b1, W2, b2, W3, b3):
    input1 = np.asarray(input1, np.float32)
    input2 = np.asarray(input2, np.float32)
    W1 = np.asarray(W1, np.float32)
    W2 = np.asarray(W2, np.float32)
    W3 = np.asarray(W3, np.float32)
    b1 = np.asarray(b1, np.float32)
    b2 = np.asarray(b2, np.float32)
    b3 = np.asarray(b3, np.float32)

    def pmajor(X, inner):
        # [THW, inner] -> [128, NI*inner]: row p = concat_n X[n*128+p, :]
        return np.ascontiguousarray(
            X.reshape(-1, 128, inner).transpose(1, 0, 2).reshape(128, -1)
        )

    # [B,T,C,H,W] -> x[b][i=(t,hw), c], partition-major
    X1f = np.ascontiguousarray(
        input1.reshape(B, T, C, H * W).transpose(0, 1, 3, 2)
    ).reshape(B, THW, C)
    X2f = np.ascontiguousarray(
        input2.reshape(B, T, C, H * W).transpose(0, 1, 3, 2)
    ).reshape(B, THW, C)
    X1g = (GAMMA * X1f).astype(FP16NP)          # gamma-scaled fp16
    X2 = X2f.astype(FP16NP)
    X1p = [pmajor(X1g[b], C) for b in range(B)]
    X2p = [pmajor(X2[b], C) for b in range(B)]
    # channel-major x1 (unscaled): [128p, (n, dt, j)] = x1[n*128+j, dt*128+p]
    X1b = X1f.astype(FP16NP)
    X1Tp = [
        np.ascontiguousarray(
            X1b[b].reshape(NI, 128, NDT, 128).transpose(3, 0, 2, 1)
        ).reshape(128, NI * NDT * 128)
        for b in range(B)
    ]
    W1Tp = pmajor(np.ascontiguousarray((W1 / GAMMA).T).astype(FP16NP), F)
    W2Tp = pmajor(np.ascontiguousarray(W2.T).astype(FP16NP), F)
    W3T = np.ascontiguousarray(W3.T)             # [THW, O_TOT]
    W3q8 = (BETA * W3T[: NF8 * 128]).astype(E4M3NP)   # fp8 rows (main ogs)
    W3bb = (BETA * W3T).astype(FP16NP)           # fp16 rows, main ogs
    W3gg = (BETA01 * W3T).astype(FP16NP)         # fp16 rows, og0/1
    b1c = np.ascontiguousarray(b1.reshape(F, 1))
    b2c = np.ascontiguousarray(b2.reshape(F, 1))

    in_maps = []
    for core in range(8):
        b = core // 2
        half = core % 2
        osl = slice(half * O_HALF, (half + 1) * O_HALF)
        W3h8 = W3q8[:, osl]
        W3hb = W3bb[:, osl]
        W3hg = W3gg[:, osl]
        # og0/1: full-depth fp16 [128, (og2, n40, oc)]
        w3g_core = np.ascontiguousarray(
            W3hg[:, : 2 * OG]
            .reshape(NI, 128, 2, OG)
            .transpose(1, 2, 0, 3)
            .reshape(128, 2 * NI * OG)
        )
        # main ogs fp8 pairs -> [128, (og, j, s, oc)]
        w3q_core = np.ascontiguousarray(
            W3h8[:, 2 * OG :]
            .reshape(P8, 2, 128, N_MOG, OG)
            .transpose(2, 3, 0, 1, 4)
            .reshape(128, N_MOG * P8 * 2 * OG)
        )
        # main ogs bf16 tail -> [128, (og, m, oc)]
        w3b_core = np.ascontiguousarray(
            W3hb[NF8 * 128 :, 2 * OG :]
            .reshape(NBF, 128, N_MOG, OG)
            .transpose(1, 2, 0, 3)
            .reshape(128, N_MOG * NBF * OG)
        )
        b3h = b3[osl]
        in_maps.append(
            {
                "x1": X1p[b],
                "x2": X2p[b],
                "x1t": X1Tp[b],
                "w1t": W1Tp,
                "w2t": W2Tp,
                "w3g": w3g_core,
                "w3q": w3q_core,
                "w3b": w3b_core,
                "b1": b1c,
                "b2": b2c,
                "b3b": np.ascontiguousarray(
                    np.broadcast_to(
                        (BETA * b3h).astype(BF16NP)[None, :], (128, O_HALF)
                    )
                ),
                "b3g": np.ascontiguousarray(
                    np.broadcast_to(
                        (GAMMA * BETA01 * b3h[: 2 * OG]).astype(FP16NP)[None, :],
                        (128, 2 * OG),
                    )
                ),
            }
        )
    return in_maps


def run(inputs: dict, trace: bool = False):
    """Returns (full_output [B,F,C,H,W], BassKernelResults)."""
    in_maps = _stage_inputs(**inputs)
    nc = _get_nc()
    res = run_bass_kernel_spmd(nc, in_maps, core_ids=list(range(8)), trace=trace)
    out_full = np.empty((B, C, O_TOT), np.float32)
    for core in range(8):
        b = core // 2
        half = core % 2
        out_full[b, :, half * O_HALF : (half + 1) * O_HALF] = res.results[core]["out"]
    # host unscale: og0/1 of each half at ALPHA*GAMMA*BETA, rest ALPHA*BETA
    inv_main = 1.0 / (ALPHA * BETA)
    inv_01 = 1.0 / (ALPHA * GAMMA * BETA01)
    for half in range(2):
        lo = half * O_HALF
        out_full[:, :, lo : lo + 2 * OG] *= inv_01
        out_full[:, :, lo + 2 * OG : lo + O_HALF] *= inv_main
    out = np.ascontiguousarray(
        out_full.reshape(B, C, F, H, W).transpose(0, 2, 1, 3, 4)
    )
    return out, res


def kernel(**inputs) -> np.ndarray:
    out, _ = run(inputs, trace=False)
    return out
